# revision 1
# baseline (speedup 1.0000x reference)
"""CenterlineLoss Trainium2 kernel.

Computes 0.5*(mean1 + mean2) where
  mean1 = mean over valid proj points of distance to nearest ref point
  mean2 = mean over ref points of distance to nearest valid proj point
(reference semantics: ref coords swapped (y,x); proj row-reversal is a
permutation and does not affect either reduction; proj validity mask
applied to both reductions).

Strategy (per spec sharding hint): shard the N=16384 proj axis across 8
cores; each core computes its [2048, 8192] squared-distance tile via
TensorE matmuls using a K=14 fp16 limb-split encoding
  d^2 = |p-c|^2 - 2(p-c).(r-c) + |r-c|^2   (c = image center, exact limbs)
accumulated in fp32 PSUM (numerically validated to ~3e-7 final rel err).
ScalarE copies PSUM->SBUF as fp16; VectorE then does both min reductions
at 2x (packed fp16): a fold tree per 128-row tile for row-mins and a
running elementwise min for column-mins (the last tile's column pass is
sliced so each colacc output DMA overlaps the remaining DVE work).  The
host finishes the cross-partition / cross-core column-min, masked means,
and sqrt in fp64.
"""

import time

import numpy as np

import concourse.bacc as bacc
import concourse.mybir as mybir
import concourse.tile as tile
from concourse import bass_utils

N = 16384
M = 8192
NCORES = 8
NLOC = N // NCORES          # 2048 proj rows per core
NTILES = NLOC // 128        # 16
K = 14                      # limb-split contraction depth
P2SCALE = 64.0
R2SCALE = 16.0
BIGVAL = 60000.0            # masked-row d^2 sentinel (fp16-exact, > any real min)
CENTER = (320.0, 240.0)

_f16 = np.float16


def _split2(v):
    h = v.astype(_f16).astype(np.float64)
    l = (v - h).astype(_f16).astype(np.float64)
    return h, l


def _split3(v):
    h = v.astype(_f16).astype(np.float64)
    r = v - h
    m = r.astype(_f16).astype(np.float64)
    l = (r - m).astype(_f16).astype(np.float64)
    return h, m, l


def _host_prep(proj_f32, ref_f32):
    proj = proj_f32.astype(np.float64)
    refs = ref_f32.astype(np.float64)[:, ::-1]  # torch flip(1): swap (x,y)

    mask = (
        (proj[:, 0] >= 0.0) & (proj[:, 0] <= 640.0)
        & (proj[:, 1] >= 0.0) & (proj[:, 1] <= 480.0)
    )

    c = np.array(CENTER)
    pt = proj - c
    rt = refs - c

    Xh, Xl = _split2(pt[:, 0])
    Yh, Yl = _split2(pt[:, 1])
    Xh_, Xl_ = _split2(rt[:, 0])
    Yh_, Yl_ = _split2(rt[:, 1])

    px, py = Xh + Xl, Yh + Yl          # the exactly-represented points
    rx, ry = Xh_ + Xl_, Yh_ + Yl_
    P2a, P2b, P2c = _split3((px * px + py * py) / P2SCALE)
    R2a, R2b, R2c = _split3((rx * rx + ry * ry) / R2SCALE)

    rs = np.full(N, R2SCALE)
    a = np.stack([Xh, Xh, Xl, Xl, Yh, Yh, Yl, Yl, P2a, P2b, P2c, rs, rs, rs])
    ps = np.full(M, P2SCALE)
    b = np.stack([-2 * Xh_, -2 * Xl_, -2 * Xh_, -2 * Xl_,
                  -2 * Yh_, -2 * Yl_, -2 * Yh_, -2 * Yl_,
                  ps, ps, ps, R2a, R2b, R2c])

    # masked proj rows: zero the row, encode constant d^2 = BIGVAL via P2a slot
    a[:, ~mask] = 0.0
    a[8, ~mask] = BIGVAL / P2SCALE

    return a.astype(_f16), b.astype(_f16), mask


_PROGRAM_CACHE = {}


def _build_program():
    if "nc" in _PROGRAM_CACHE:
        return _PROGRAM_CACHE["nc"]

    f16 = mybir.dt.float16
    f32 = mybir.dt.float32
    MIN = mybir.AluOpType.min

    nc = bacc.Bacc("TRN2", target_bir_lowering=False, debug=False,
                   num_devices=NCORES)

    a_dram = nc.dram_tensor("a_in", [K, NLOC], f16, kind="ExternalInput").ap()
    b_dram = nc.dram_tensor("b_in", [K, M], f16, kind="ExternalInput").ap()
    id_dram = nc.dram_tensor("ident_in", [128, 128], f16,
                             kind="ExternalInput").ap()
    rowm_dram = nc.dram_tensor("rowmin_out", [128, NTILES], f32,
                               kind="ExternalOutput").ap()
    colm_dram = nc.dram_tensor("colacc_out", [128, M], f16,
                               kind="ExternalOutput").ap()

    with tile.TileContext(nc) as tc, \
            tc.tile_pool(name="const", bufs=1) as const_pool:
        a_sb = const_pool.tile([K, NLOC], f16, tag="a_sb")
        b_sb = const_pool.tile([K, M], f16, tag="b_sb")
        id_sb = const_pool.tile([128, 128], f16, tag="id_sb")
        colacc = const_pool.tile([128, M], f16, tag="colacc")
        rowm = const_pool.tile([128, NTILES], f32, tag="rowm")
        colm = const_pool.tile([128, M // 128], f32, tag="colm")

        s6cat = const_pool.tile([128, 512], f16, tag="s6cat")
        warm = const_pool.tile([1, 8], f16, tag="warm")

        # trigger the ACT function-table load while DMAs are in flight
        nc.scalar.copy(warm[:, 4:], warm[:, :4])
        nc.sync.dma_start(a_sb[:], a_dram)
        # split b so the first matmuls can start before the whole tensor lands
        for h in range(4):
            nc.gpsimd.dma_start(b_sb[:, h * 2048:(h + 1) * 2048],
                                b_dram[:, h * 2048:(h + 1) * 2048])
        nc.sync.dma_start(id_sb[:], id_dram)

        with (
            tc.tile_pool(name="mmpsum", bufs=2, space="PSUM") as psum_pool,
            tc.tile_pool(name="data", bufs=3) as data_pool,
            tc.tile_pool(name="fold", bufs=2) as fold_pool,
        ):
            for t in range(NTILES):
                lhsT = a_sb[:, t * 128:(t + 1) * 128]
                dtile = data_pool.tile([128, M], f16, tag="dtile")
                # 4 PSUM rounds of 2048 cols (4 banks each, double-buffered).
                # Tile 0 fills quarters in order 0,2,1,3 so DVE can start the
                # fold/col work after only two ScalarE copies (shorter head).
                jorder = (0, 2, 1, 3) if t == 0 else (0, 1, 2, 3)
                for j in jorder:
                    ps_t = psum_pool.tile([128, 2048], f32, tag="mm")
                    for q in range(4):
                        cc = j * 4 + q
                        nc.tensor.matmul(
                            ps_t[:, q * 512:(q + 1) * 512],
                            lhsT,
                            b_sb[:, cc * 512:(cc + 1) * 512],
                            start=True, stop=True,
                        )
                    # fp32 PSUM -> fp16 SBUF (ScalarE)
                    nc.scalar.copy(dtile[:, j * 2048:(j + 1) * 2048], ps_t[:])

                s1 = fold_pool.tile([128, 4096], f16, tag="s1")
                if t == 0:
                    # seed colacc via 4x copies (cheaper than memset + TT) and
                    # run the first fold level in halves as quarters land
                    nc.vector.tensor_tensor(s1[:, :2048], dtile[:, :2048],
                                            dtile[:, 4096:6144], op=MIN)
                    nc.vector.tensor_copy(colacc[:, :4096], dtile[:, :4096])
                    nc.vector.tensor_tensor(s1[:, 2048:], dtile[:, 2048:4096],
                                            dtile[:, 6144:], op=MIN)
                    nc.vector.tensor_copy(colacc[:, 4096:], dtile[:, 4096:])
                elif t == NTILES - 1:
                    # last tile: finish colacc in column slices so each
                    # output DMA overlaps the remaining DVE work
                    for sl in range(4):
                        cr = slice(sl * 2048, (sl + 1) * 2048)
                        nc.vector.tensor_tensor(colacc[:, cr], dtile[:, cr],
                                                colacc[:, cr], op=MIN)
                        nc.sync.dma_start(colm_dram[:, cr], colacc[:, cr])
                    nc.vector.tensor_tensor(s1[:], dtile[:, :4096],
                                            dtile[:, 4096:], op=MIN)
                else:
                    # column minima: running elementwise min (2x packed fp16)
                    nc.vector.tensor_tensor(colacc[:], dtile[:], colacc[:],
                                            op=MIN)
                    # row minima: fold tree (2x packed fp16), batching the
                    # last levels of 4 consecutive tiles into one strided op
                    nc.vector.tensor_tensor(s1[:], dtile[:, :4096],
                                            dtile[:, 4096:], op=MIN)
                s2 = fold_pool.tile([128, 2048], f16, tag="s2")
                nc.vector.tensor_tensor(s2[:], s1[:, :2048], s1[:, 2048:],
                                        op=MIN)
                s3 = fold_pool.tile([128, 1024], f16, tag="s3")
                nc.vector.tensor_tensor(s3[:], s2[:, :1024], s2[:, 1024:],
                                        op=MIN)
                s4 = fold_pool.tile([128, 512], f16, tag="s4")
                nc.vector.tensor_tensor(s4[:], s3[:, :512], s3[:, 512:],
                                        op=MIN)
                s5 = fold_pool.tile([128, 256], f16, tag="s5")
                nc.vector.tensor_tensor(s5[:], s4[:, :256], s4[:, 256:],
                                        op=MIN)
                u = t % 4
                nc.vector.tensor_tensor(s6cat[:, u * 128:(u + 1) * 128],
                                        s5[:, :128], s5[:, 128:], op=MIN)
                if u == 3:
                    cat3 = s6cat[:].rearrange("p (b f) -> p b f", f=128)
                    nc.vector.tensor_reduce(rowm[:, t - 3:t + 1], cat3,
                                            axis=mybir.AxisListType.X, op=MIN)

        nc.sync.dma_start(rowm_dram, rowm[:])

    nc.compile()
    _PROGRAM_CACHE["nc"] = nc
    return nc


def _run_on_hw(a, b, trace=False, tmpdir=None):
    nc = _build_program()
    ident = np.eye(128, dtype=_f16)
    in_maps = [
        {
            "a_in": np.ascontiguousarray(a[:, c * NLOC:(c + 1) * NLOC]),
            "b_in": b,
            "ident_in": ident,
        }
        for c in range(NCORES)
    ]
    # transient NRT_EXEC_UNIT_UNRECOVERABLE states clear after the worker
    # recycles; retry with increasing waits
    last = None
    for wait_s in (0, 30, 60, 90):
        if wait_s:
            time.sleep(wait_s)
        try:
            return bass_utils.run_bass_kernel_spmd(
                nc, in_maps, core_ids=list(range(NCORES)), trace=trace,
                tmpdir=tmpdir,
            )
        except Exception as e:
            last = e
    raise last


def kernel(bezier_proj_centerline_img, ref_catheter_centerline, _trace=False,
           _tmpdir=None):
    a, b, mask = _host_prep(
        np.asarray(bezier_proj_centerline_img, dtype=np.float32),
        np.asarray(ref_catheter_centerline, dtype=np.float32),
    )

    res = _run_on_hw(a, b, trace=_trace, tmpdir=_tmpdir)

    rowmins = np.empty(N, np.float64)
    colmin = np.full(M, np.inf)
    for c in range(NCORES):
        out = res.results[c]
        rm = out["rowmin_out"].astype(np.float64)      # [128, NTILES]
        ca = out["colacc_out"].astype(np.float32)      # [128, M]
        rowmins[c * NLOC:(c + 1) * NLOC] = rm.T.reshape(-1)
        colmin = np.minimum(colmin, ca.min(axis=0).astype(np.float64))

    mean1 = np.sqrt(np.maximum(rowmins[mask], 0.0)).mean()
    mean2 = np.sqrt(np.maximum(colmin, 0.0)).mean()
    out = np.float32(0.5 * (mean1 + mean2))
    if _trace:
        return out, res
    return out



# revision 19
# speedup vs baseline: 6.8169x; 6.8169x over previous
"""CenterlineLoss Trainium2 kernel — windowed two-pass nearest-neighbor.

Computes 0.5*(mean1 + mean2) where
  mean1 = mean over valid proj points of distance to nearest ref point
  mean2 = mean over ref points of distance to nearest valid proj point
(reference semantics: ref coords swapped; proj row order irrelevant;
proj validity mask applied to both reductions).

Strategy: the all-pairs [N, M] distance matrix is never materialized.
Host sorts the valid proj points and the refs along x and gathers, for
every 128-point tile, a contiguous 512-wide candidate window from the
other (sorted) point set.  The device computes, per tile, a [128, 512]
squared-distance block via one TensorE matmul (K=14 fp16 limb-split
encoding, d^2 exact to fp32) and reduces it to per-row minima with a
fused tensor_tensor_reduce — work is split across ACT/Pool/DVE so all
engines stay busy.  Refs whose y lies beyond the proj y-extent get
their candidates from a boundary band of proj sorted by x instead.

Correctness does not depend on the windows: the host computes, per
query row, a lower bound on the distance to any EXCLUDED candidate
(x-gap to the window edge, y-clearance to the set extent, band bound).
Rows whose found min does not beat that bound are recomputed exactly
on the host (typically 0-2 rows).  Degenerate inputs (few valid
points) fall back to an exact host computation.
"""

import time

import numpy as np

import concourse.bacc as bacc
import concourse.mybir as mybir
import concourse.tile as tile
from concourse import bass_utils

N = 16384
M = 8192
NCORES = 8
K = 14                      # limb-split contraction depth
CP = 512                    # candidate window width per 128-row tile
P2SCALE = 64.0
R2SCALE = 16.0
BIGVAL = 60000.0            # sentinel d^2 (> any real window min)
TTR_INIT = 1.0e30           # accumulator init for the min-reduce
CENTER = (320.0, 240.0)
BAND_W = 48.0               # boundary-band depth for far refs
TAU = 2.0                   # y-clearance above which a ref is "far"
NWARM = 10                  # dummy matmuls to ramp the PE p-state
REF_TILES = M // 128 // NCORES  # 8 ref tiles per core

_f16 = np.float16


def _split2(v):
    h = v.astype(_f16).astype(np.float64)
    l = (v - h).astype(_f16).astype(np.float64)
    return h, l


def _split3(v):
    h = v.astype(_f16).astype(np.float64)
    r = v - h
    m = r.astype(_f16).astype(np.float64)
    l = (r - m).astype(_f16).astype(np.float64)
    return h, m, l


def _enc_a(pts):
    """Row-side limb encoding (points on the partition axis). [n,2]->[K,n]"""
    x = pts[:, 0]
    y = pts[:, 1]
    Xh, Xl = _split2(x)
    Yh, Yl = _split2(y)
    px, py = Xh + Xl, Yh + Yl
    P2a, P2b, P2c = _split3((px * px + py * py) / P2SCALE)
    rs = np.full(len(x), R2SCALE)
    return np.stack(
        [Xh, Xh, Xl, Xl, Yh, Yh, Yl, Yl, P2a, P2b, P2c, rs, rs, rs]
    ).astype(_f16)


def _enc_b(pts):
    """Column-side limb encoding (candidate points). [n,2]->[K,n]"""
    x = pts[:, 0]
    y = pts[:, 1]
    Xh, Xl = _split2(x)
    Yh, Yl = _split2(y)
    rx, ry = Xh + Xl, Yh + Yl
    R2a, R2b, R2c = _split3((rx * rx + ry * ry) / R2SCALE)
    ps = np.full(len(x), P2SCALE)
    return np.stack(
        [-2 * Xh, -2 * Xl, -2 * Xh, -2 * Xl,
         -2 * Yh, -2 * Yl, -2 * Yh, -2 * Yl,
         ps, ps, ps, R2a, R2b, R2c]
    ).astype(_f16)


_B_SENT = None


def _b_sentinel():
    """Candidate-side sentinel column: d^2 == BIGVAL against any row."""
    global _B_SENT
    if _B_SENT is None:
        col = np.zeros((K, 1), _f16)
        col[11, 0] = _f16(BIGVAL / R2SCALE)
        _B_SENT = col
    return _B_SENT


_PROGRAM_CACHE = {}

USE_TTR = False    # TENSOR_TENSOR_REDUCE with op=min crashes the exec unit
USE_WARM = True    # debug knob: False -> no PE warm-up block

# per-tile reduction modes (GPSIMD cannot run TensorTensor or touch PSUM;
# DVE may read at most one PSUM operand per instruction):
#   0: DVE tensor_reduce directly from PSUM         [DVE ~658ns]
#   1: ACT evacuate to SBUF; DVE fused TTR          [ACT ~612, DVE ~327]
# mix balances ACT vs DVE busy time
_MODE_PATTERN = (0,)


def _build_program(T_p=14):
    key = (T_p, USE_TTR, USE_WARM)
    if key in _PROGRAM_CACHE:
        return _PROGRAM_CACHE[key]

    f16 = mybir.dt.float16
    f32 = mybir.dt.float32
    MIN = mybir.AluOpType.min

    nc = bacc.Bacc("TRN2", target_bir_lowering=False, debug=False,
                   num_devices=NCORES)

    WAB = T_p * (128 + CP)
    WCD = REF_TILES * (128 + CP)
    ab_dram = nc.dram_tensor("ab_in", [K, WAB], f16, kind="ExternalInput").ap()
    cd_dram = nc.dram_tensor("cd_in", [K, WCD], f16, kind="ExternalInput").ap()
    rowm_dram = nc.dram_tensor("rowm_out", [128, T_p], f32,
                               kind="ExternalOutput").ap()
    refm_dram = nc.dram_tensor("refm_out", [128, REF_TILES], f32,
                               kind="ExternalOutput").ap()

    with tile.TileContext(nc) as tc, \
            tc.tile_pool(name="const", bufs=1) as cpool:
        ab_sb = cpool.tile([K, WAB], f16, tag="ab")
        cd_sb = cpool.tile([K, WCD], f16, tag="cd")
        wsrc = cpool.tile([K, CP], f16, tag="wsrc")
        rowm = cpool.tile([128, T_p], f32, tag="rowm")
        refm = cpool.tile([128, REF_TILES], f32, tag="refm")

        # inputs on two separate queues so both are in flight immediately
        nc.sync.dma_start(ab_sb[:], ab_dram)
        nc.scalar.dma_start(cd_sb[:], cd_dram)

        with (
            tc.tile_pool(name="wps", bufs=1, space="PSUM") as wpool,
            tc.tile_pool(name="mm", bufs=4, space="PSUM") as pspool,
            tc.tile_pool(name="evac", bufs=2) as epool,
            tc.tile_pool(name="s1", bufs=3) as s1pool,
            tc.tile_pool(name="s2", bufs=2) as s2pool,
        ):
            # dummy matmuls during the input DMAs keep the PE busy so the
            # p-state is fully ramped when the real tiles start
            if USE_WARM:
                wps = wpool.tile([128, CP], f32, tag="wps")
                nc.gpsimd.memset(wsrc[:], 0.0)
                for _ in range(NWARM):
                    nc.tensor.matmul(wps[:], wsrc[:, :128], wsrc[:],
                                     start=True, stop=True)

            H = CP // 2
            Q = CP // 4

            def ttr(out, src, w, acc):
                if USE_TTR:
                    nc.vector.tensor_tensor_reduce(
                        out.broadcast_to(src[:, :w].shape),
                        src[:, :w], src[:, w:2 * w], scale=1.0,
                        scalar=4.0 * BIGVAL, op0=MIN, op1=MIN,
                        accum_out=acc)
                else:
                    nc.vector.tensor_reduce(acc, src[:, :2 * w], op=MIN,
                                            axis=mybir.AxisListType.X)

            def do_tile(i, lhsT, rhs, acc):
                ps = pspool.tile([128, CP], f32, tag="mm")
                nc.tensor.matmul(ps[:], lhsT, rhs, start=True, stop=True)
                mode = _MODE_PATTERN[i % len(_MODE_PATTERN)]
                if mode == 0:
                    nc.vector.tensor_reduce(acc, ps[:], op=MIN,
                                            axis=mybir.AxisListType.X)
                    return
                dt = epool.tile([128, CP], f32, tag="dt")
                nc.scalar.copy(dt[:], ps[:])
                s1 = s1pool.tile([128, 1], f32, tag="s1")
                ttr(s1, dt, H, acc)

            for t in range(T_p):
                do_tile(t, ab_sb[:, t * 128:(t + 1) * 128],
                        ab_sb[:, T_p * 128 + t * CP:T_p * 128 + (t + 1) * CP],
                        rowm[:, t:t + 1])
            nc.sync.dma_start(rowm_dram, rowm[:])
            for u in range(REF_TILES):
                do_tile(T_p + u, cd_sb[:, u * 128:(u + 1) * 128],
                        cd_sb[:, REF_TILES * 128 + u * CP:
                              REF_TILES * 128 + (u + 1) * CP],
                        refm[:, u:u + 1])
            nc.sync.dma_start(refm_dram, refm[:])

    nc.compile()
    _PROGRAM_CACHE[T_p] = nc
    return nc


def _gather_windows(enc, n_real, offs):
    """Stack enc[:, o:o+CP] slices; pad short sources with sentinels."""
    cols = []
    for o in offs:
        if n_real >= CP:
            cols.append(enc[:, o:o + CP])
        else:
            pad = np.broadcast_to(_b_sentinel(), (K, CP - n_real))
            cols.append(np.concatenate([enc[:, :n_real], pad], axis=1))
    return np.concatenate(cols, axis=1)


def _window_offsets(tile_lo_x, tile_hi_x, cand_x, n_cand):
    ja = np.searchsorted(cand_x, tile_lo_x)
    jb = np.searchsorted(cand_x, tile_hi_x)
    return int(np.clip((ja + jb) // 2 - CP // 2, 0, max(0, n_cand - CP)))


def _edge_margins(qx, yclear, cand_x, n_cand, o):
    """Min distance from query rows to any candidate excluded by the
    x-window [o, o+CP) — hypot of x-gap past the nearest excluded
    element and the y-clearance to the candidate set's y-extent."""
    n = len(qx)
    if o > 0:
        ml = np.hypot(np.maximum(qx - cand_x[o - 1], 0.0), yclear)
    else:
        ml = np.full(n, np.inf)
    if o + CP < n_cand:
        mr = np.hypot(np.maximum(cand_x[o + CP] - qx, 0.0), yclear)
    else:
        mr = np.full(n, np.inf)
    return np.minimum(ml, mr)


def _run_on_hw(in_maps, T_p, trace=False, tmpdir=None):
    nc = _build_program(T_p)
    last = None
    for wait_s in (0, 30, 60, 90):
        if wait_s:
            time.sleep(wait_s)
        try:
            return bass_utils.run_bass_kernel_spmd(
                nc, in_maps, core_ids=list(range(NCORES)), trace=trace,
                tmpdir=tmpdir,
            )
        except Exception as e:
            last = e
    raise last


def kernel(bezier_proj_centerline_img, ref_catheter_centerline, _trace=False,
           _tmpdir=None):
    proj = np.asarray(bezier_proj_centerline_img, np.float64)
    refs_all = np.asarray(ref_catheter_centerline, np.float64)[:, ::-1]
    c = np.array(CENTER)

    mask = (
        (proj[:, 0] >= 0.0) & (proj[:, 0] <= 640.0)
        & (proj[:, 1] >= 0.0) & (proj[:, 1] <= 480.0)
    )
    pv = proj[mask]
    nv = len(pv)
    m_ref = len(refs_all)

    if nv < 2 * CP or m_ref != M:
        # degenerate input: exact host computation
        if nv == 0:
            mean1 = np.nan
            mean2 = np.sqrt(((refs_all[:, None, :] - proj[None, :, :]) ** 2)
                            .sum(-1)).min(1).mean() if len(proj) else np.nan
            out = np.float32(0.5 * (mean1 + mean2))
        else:
            d2 = ((pv[:, None, :] - refs_all[None, :, :]) ** 2).sum(-1)
            mean1 = np.sqrt(d2.min(1)).mean()
            mean2 = np.sqrt(d2.min(0)).mean()
            out = np.float32(0.5 * (mean1 + mean2))
        if _trace:
            return out, None
        return out

    pvs = pv[np.argsort(pv[:, 0], kind="stable")] - c
    px = pvs[:, 0]
    py_lo, py_hi = pvs[:, 1].min(), pvs[:, 1].max()
    rsx = refs_all[np.argsort(refs_all[:, 0], kind="stable")] - c
    rx = rsx[:, 0]

    R_pc = int(np.ceil(nv / (NCORES * 128))) * 128
    NP = NCORES * R_pc
    T_p = R_pc // 128
    T_tot = NP // 128

    # ---- proj-side pass: rows = sorted valid proj, candidates = refs ----
    A = np.concatenate([_enc_a(pvs), np.zeros((K, NP - nv), _f16)], axis=1)
    A[8, nv:] = _f16(BIGVAL / P2SCALE)
    B = _enc_b(rsx)

    p_offs = np.zeros(T_tot, np.int64)
    for g in range(T_tot):
        lo, hi = 128 * g, min(128 * (g + 1), nv)
        if lo >= nv:
            continue
        p_offs[g] = _window_offsets(px[lo], px[hi - 1], rx, M)
    bgath = _gather_windows(B, M, p_offs)

    # ---- ref-side pass: rows = refs (class-ordered), candidates = proj ----
    far_top = rsx[:, 1] > py_hi + TAU
    far_bot = rsx[:, 1] < py_lo - TAU
    near_i = np.where(~(far_top | far_bot))[0]
    n_keep = (len(near_i) // 128) * 128
    if n_keep < len(near_i):
        by_y = near_i[np.argsort(np.abs(rsx[near_i][:, 1]), kind="stable")]
        keep, movers = by_y[:n_keep], by_y[n_keep:]
    else:
        keep, movers = near_i, np.array([], np.int64)
    top_i = np.concatenate([np.where(far_top)[0], movers]).astype(np.int64)
    bot_i = np.where(far_bot)[0]
    ordr = np.concatenate([
        keep[np.argsort(rsx[keep][:, 0], kind="stable")],
        top_i[np.argsort(rsx[top_i][:, 0], kind="stable")],
        bot_i[np.argsort(rsx[bot_i][:, 0], kind="stable")],
    ])
    rs2 = rsx[ordr]
    n_near, n_top = len(keep), len(top_i)

    band_t = np.where(pvs[:, 1] >= py_hi - BAND_W)[0]
    band_b = np.where(pvs[:, 1] <= py_lo + BAND_W)[0]
    btx = pvs[band_t][:, 0]
    bbx = pvs[band_b][:, 0]
    BT = _enc_b(pvs[band_t]) if len(band_t) else np.zeros((K, 0), _f16)
    BB = _enc_b(pvs[band_b]) if len(band_b) else np.zeros((K, 0), _f16)

    AT = _enc_a(rs2)
    r_offs = np.zeros(M // 128, np.int64)
    r_kind = [None] * (M // 128)
    for u in range(M // 128):
        lo, hi = 128 * u, 128 * (u + 1)
        if hi <= n_near:
            kind = "near"
        elif lo >= n_near and hi <= n_near + n_top:
            kind = "top"
        elif lo >= n_near + n_top:
            kind = "bot"
        else:
            kind = "top" if (hi - n_near) > 64 and len(btx) else "near"
            if lo >= n_near and len(bbx) and (hi - (n_near + n_top)) > 64:
                kind = "bot"
        r_kind[u] = kind
        cx = {"near": px, "top": btx, "bot": bbx}[kind]
        xlo, xhi = rs2[lo:hi, 0].min(), rs2[lo:hi, 0].max()
        r_offs[u] = _window_offsets(xlo, xhi, cx, len(cx))
    BPm = _enc_b(pvs)
    src = {"near": (BPm, nv), "top": (BT, len(btx)), "bot": (BB, len(bbx))}
    bpg = np.concatenate([
        _gather_windows(src[r_kind[u]][0], src[r_kind[u]][1], [r_offs[u]])
        for u in range(M // 128)
    ], axis=1)

    # ---- run on hardware ----
    in_maps = []
    for cc in range(NCORES):
        ab = np.concatenate([
            A[:, cc * R_pc:(cc + 1) * R_pc],
            bgath[:, cc * T_p * CP:(cc + 1) * T_p * CP],
        ], axis=1)
        cd = np.concatenate([
            AT[:, cc * REF_TILES * 128:(cc + 1) * REF_TILES * 128],
            bpg[:, cc * REF_TILES * CP:(cc + 1) * REF_TILES * CP],
        ], axis=1)
        in_maps.append({"ab_in": np.ascontiguousarray(ab),
                        "cd_in": np.ascontiguousarray(cd)})

    res = _run_on_hw(in_maps, T_p, trace=_trace, tmpdir=_tmpdir)

    rowd2 = np.empty(NP)
    refd2 = np.empty(M)
    for cc in range(NCORES):
        out = res.results[cc]
        rowd2[cc * R_pc:(cc + 1) * R_pc] = \
            out["rowm_out"].astype(np.float64).T.reshape(-1)
        refd2[cc * REF_TILES * 128:(cc + 1) * REF_TILES * 128] = \
            out["refm_out"].astype(np.float64).T.reshape(-1)

    # ---- host: margins, fallback, means ----
    ry_lo, ry_hi = rsx[:, 1].min(), rsx[:, 1].max()
    found1 = np.sqrt(np.maximum(rowd2[:nv], 0.0))
    yc1 = np.maximum(0.0, np.maximum(pvs[:, 1] - ry_hi, ry_lo - pvs[:, 1]))
    marg1 = np.full(nv, np.inf)
    for g in range((nv + 127) // 128):
        lo, hi = 128 * g, min(128 * (g + 1), nv)
        marg1[lo:hi] = _edge_margins(px[lo:hi], yc1[lo:hi], rx, M,
                                     int(p_offs[g]))
    slack1 = np.maximum(1e-3 * found1, 1e-4)
    bad1 = (found1 > marg1 - slack1) | ~np.isfinite(found1)
    if bad1.any():
        ii = np.where(bad1)[0]
        d2x = ((pvs[ii, None, :] - rsx[None, :, :]) ** 2).sum(-1).min(1)
        found1[ii] = np.sqrt(d2x)
    mean1 = found1.mean()

    found2 = np.sqrt(np.maximum(refd2, 0.0))
    yc2 = np.maximum(0.0, np.maximum(rs2[:, 1] - py_hi, py_lo - rs2[:, 1]))
    marg2 = np.full(M, np.inf)
    for u in range(M // 128):
        lo, hi = 128 * u, 128 * (u + 1)
        kind = r_kind[u]
        cx = {"near": px, "top": btx, "bot": bbx}[kind]
        m = _edge_margins(rs2[lo:hi, 0], yc2[lo:hi], cx, len(cx),
                          int(r_offs[u]))
        if kind == "top":
            m = np.minimum(np.maximum(rs2[lo:hi, 1] - (py_hi - BAND_W), 0.0),
                           m)
        elif kind == "bot":
            m = np.minimum(np.maximum((py_lo + BAND_W) - rs2[lo:hi, 1], 0.0),
                           m)
        marg2[lo:hi] = m
    slack2 = np.maximum(1e-3 * found2, 1e-4)
    bad2 = (found2 > marg2 - slack2) | ~np.isfinite(found2)
    if bad2.any():
        jj = np.where(bad2)[0]
        d2x = ((rs2[jj, None, :] - pvs[None, :, :]) ** 2).sum(-1).min(1)
        found2[jj] = np.sqrt(d2x)
    mean2 = found2.mean()

    out = np.float32(0.5 * (mean1 + mean2))
    if _trace:
        return out, res
    return out


# revision 23
# speedup vs baseline: 8.7813x; 1.2882x over previous
"""CenterlineLoss Trainium2 kernel — windowed two-pass nearest-neighbor.

Computes 0.5*(mean1 + mean2) where
  mean1 = mean over valid proj points of distance to nearest ref point
  mean2 = mean over ref points of distance to nearest valid proj point
(reference semantics: ref coords swapped; proj row order irrelevant;
proj validity mask applied to both reductions).

Strategy: the all-pairs [N, M] distance matrix is never materialized.
Host sorts the valid proj points and the refs along x and gathers, for
every 128-point tile, a contiguous 512-wide candidate window from the
other (sorted) point set.  The device computes, per tile, a [128, 512]
squared-distance block via one TensorE matmul (K=14 fp16 limb-split
encoding, d^2 exact to fp32) and reduces it to per-row minima with a
fused tensor_tensor_reduce — work is split across ACT/Pool/DVE so all
engines stay busy.  Refs whose y lies beyond the proj y-extent get
their candidates from a boundary band of proj sorted by x instead.

Correctness does not depend on the windows: the host computes, per
query row, a lower bound on the distance to any EXCLUDED candidate
(x-gap to the window edge, y-clearance to the set extent, band bound).
Rows whose found min does not beat that bound are recomputed exactly
on the host (typically 0-2 rows).  Degenerate inputs (few valid
points) fall back to an exact host computation.
"""

import time

import numpy as np

import concourse.bacc as bacc
import concourse.mybir as mybir
import concourse.tile as tile
from concourse import bass_utils

N = 16384
M = 8192
NCORES = 8
K = 14                      # limb-split contraction depth
CP = 384                    # proj-side candidate window per 128-row tile
CR = 512                    # ref-side candidate window per 128-row tile
P2SCALE = 64.0
R2SCALE = 16.0
BIGVAL = 60000.0            # sentinel d^2 (> any real window min)
TTR_INIT = 1.0e30           # accumulator init for the min-reduce
CENTER = (320.0, 240.0)
BAND_W = 48.0               # boundary-band depth for far refs
TAU = 2.0                   # y-clearance above which a ref is "far"
NWARM = 10                  # dummy matmuls to ramp the PE p-state
REF_TILES = M // 128 // NCORES  # 8 ref tiles per core

_f16 = np.float16


def _split2(v):
    h = v.astype(_f16).astype(np.float64)
    l = (v - h).astype(_f16).astype(np.float64)
    return h, l


def _split3(v):
    h = v.astype(_f16).astype(np.float64)
    r = v - h
    m = r.astype(_f16).astype(np.float64)
    l = (r - m).astype(_f16).astype(np.float64)
    return h, m, l


def _enc_a(pts):
    """Row-side limb encoding (points on the partition axis). [n,2]->[K,n]"""
    x = pts[:, 0]
    y = pts[:, 1]
    Xh, Xl = _split2(x)
    Yh, Yl = _split2(y)
    px, py = Xh + Xl, Yh + Yl
    P2a, P2b, P2c = _split3((px * px + py * py) / P2SCALE)
    rs = np.full(len(x), R2SCALE)
    return np.stack(
        [Xh, Xh, Xl, Xl, Yh, Yh, Yl, Yl, P2a, P2b, P2c, rs, rs, rs]
    ).astype(_f16)


def _enc_b(pts):
    """Column-side limb encoding (candidate points). [n,2]->[K,n]"""
    x = pts[:, 0]
    y = pts[:, 1]
    Xh, Xl = _split2(x)
    Yh, Yl = _split2(y)
    rx, ry = Xh + Xl, Yh + Yl
    R2a, R2b, R2c = _split3((rx * rx + ry * ry) / R2SCALE)
    ps = np.full(len(x), P2SCALE)
    return np.stack(
        [-2 * Xh, -2 * Xl, -2 * Xh, -2 * Xl,
         -2 * Yh, -2 * Yl, -2 * Yh, -2 * Yl,
         ps, ps, ps, R2a, R2b, R2c]
    ).astype(_f16)


_B_SENT = None


def _b_sentinel():
    """Candidate-side sentinel column: d^2 == BIGVAL against any row."""
    global _B_SENT
    if _B_SENT is None:
        col = np.zeros((K, 1), _f16)
        col[11, 0] = _f16(BIGVAL / R2SCALE)
        _B_SENT = col
    return _B_SENT


_PROGRAM_CACHE = {}

USE_TTR = False    # TENSOR_TENSOR_REDUCE with op=min crashes the exec unit
USE_WARM = True    # debug knob: False -> no PE warm-up block

# per-tile reduction modes (GPSIMD cannot run TensorTensor or touch PSUM;
# DVE may read at most one PSUM operand per instruction):
#   0: DVE tensor_reduce directly from PSUM         [DVE ~658ns]
#   1: ACT evacuate to SBUF; DVE fused TTR          [ACT ~612, DVE ~327]
# mix balances ACT vs DVE busy time
_MODE_PATTERN = (0,)


def _build_program(T_p=14):
    key = (T_p, USE_TTR, USE_WARM)
    if key in _PROGRAM_CACHE:
        return _PROGRAM_CACHE[key]

    f16 = mybir.dt.float16
    f32 = mybir.dt.float32
    MIN = mybir.AluOpType.min

    nc = bacc.Bacc("TRN2", target_bir_lowering=False, debug=False,
                   num_devices=NCORES)

    WAB = T_p * (128 + CP)
    WCD = REF_TILES * (128 + CR)
    ab_dram = nc.dram_tensor("ab_in", [K, WAB], f16, kind="ExternalInput").ap()
    cd_dram = nc.dram_tensor("cd_in", [K, WCD], f16, kind="ExternalInput").ap()
    rowm_dram = nc.dram_tensor("rowm_out", [128, T_p], f32,
                               kind="ExternalOutput").ap()
    refm_dram = nc.dram_tensor("refm_out", [128, REF_TILES], f32,
                               kind="ExternalOutput").ap()

    with tile.TileContext(nc) as tc, \
            tc.tile_pool(name="const", bufs=1) as cpool:
        ab_sb = cpool.tile([K, WAB], f16, tag="ab")
        cd_sb = cpool.tile([K, WCD], f16, tag="cd")
        rowm = cpool.tile([128, T_p], f32, tag="rowm")
        refm = cpool.tile([128, REF_TILES], f32, tag="refm")

        # inputs on two separate queues so both are in flight immediately
        nc.sync.dma_start(ab_sb[:], ab_dram)
        nc.scalar.dma_start(cd_sb[:], cd_dram)

        # matmul tiles are grouped 4-per-PSUM-allocation (at 512-col
        # stride = one bank per tile) so a single strided tensor_reduce
        # [128, g, w] -> [128, g] retires a whole group
        with tc.tile_pool(name="mm", bufs=2, space="PSUM") as pspool:

            def do_group(tiles, a_base, b_base, w, acc, res_sb):
                ps = pspool.tile([128, 2048], f32, tag="mm")
                for k, t in enumerate(tiles):
                    nc.tensor.matmul(
                        ps[:, k * 512:k * 512 + w],
                        res_sb[:, a_base + t * 128:a_base + (t + 1) * 128],
                        res_sb[:, b_base + t * w:b_base + (t + 1) * w],
                        start=True, stop=True)
                view = ps[:].rearrange("p (b f) -> p b f", f=512)
                nc.vector.tensor_reduce(
                    acc, view[:, :len(tiles), :w], op=MIN,
                    axis=mybir.AxisListType.X)

            for t0 in range(0, T_p, 4):
                tiles = list(range(t0, min(t0 + 4, T_p)))
                do_group(tiles, 0, T_p * 128, CP,
                         rowm[:, t0:t0 + len(tiles)], ab_sb)
            nc.sync.dma_start(rowm_dram, rowm[:])
            for u0 in range(0, REF_TILES, 4):
                tiles = list(range(u0, min(u0 + 4, REF_TILES)))
                do_group(tiles, 0, REF_TILES * 128, CR,
                         refm[:, u0:u0 + len(tiles)], cd_sb)
            nc.sync.dma_start(refm_dram, refm[:])

    nc.compile()
    _PROGRAM_CACHE[key] = nc
    return nc


def _gather_windows(enc, n_real, offs, w):
    """Stack enc[:, o:o+w] slices; pad short sources with sentinels."""
    cols = []
    for o in offs:
        if n_real >= w:
            cols.append(enc[:, o:o + w])
        else:
            pad = np.broadcast_to(_b_sentinel(), (K, w - n_real))
            cols.append(np.concatenate([enc[:, :n_real], pad], axis=1))
    return np.concatenate(cols, axis=1)


def _window_offsets(tile_lo_x, tile_hi_x, cand_x, n_cand, w):
    ja = np.searchsorted(cand_x, tile_lo_x)
    jb = np.searchsorted(cand_x, tile_hi_x)
    return int(np.clip((ja + jb) // 2 - w // 2, 0, max(0, n_cand - w)))


def _edge_margins(qx, yclear, cand_x, n_cand, o, w):
    """Min distance from query rows to any candidate excluded by the
    x-window [o, o+w) — hypot of x-gap past the nearest excluded
    element and the y-clearance to the candidate set's y-extent."""
    n = len(qx)
    if o > 0:
        ml = np.hypot(np.maximum(qx - cand_x[o - 1], 0.0), yclear)
    else:
        ml = np.full(n, np.inf)
    if o + w < n_cand:
        mr = np.hypot(np.maximum(cand_x[o + w] - qx, 0.0), yclear)
    else:
        mr = np.full(n, np.inf)
    return np.minimum(ml, mr)


def _run_on_hw(in_maps, T_p, trace=False, tmpdir=None):
    nc = _build_program(T_p)
    last = None
    for wait_s in (0, 30, 60, 90):
        if wait_s:
            time.sleep(wait_s)
        try:
            return bass_utils.run_bass_kernel_spmd(
                nc, in_maps, core_ids=list(range(NCORES)), trace=trace,
                tmpdir=tmpdir,
            )
        except Exception as e:
            last = e
    raise last


def kernel(bezier_proj_centerline_img, ref_catheter_centerline, _trace=False,
           _tmpdir=None):
    proj = np.asarray(bezier_proj_centerline_img, np.float64)
    refs_all = np.asarray(ref_catheter_centerline, np.float64)[:, ::-1]
    c = np.array(CENTER)

    mask = (
        (proj[:, 0] >= 0.0) & (proj[:, 0] <= 640.0)
        & (proj[:, 1] >= 0.0) & (proj[:, 1] <= 480.0)
    )
    pv = proj[mask]
    nv = len(pv)
    m_ref = len(refs_all)

    if nv < 2 * CP or m_ref != M:
        # degenerate input: exact host computation
        if nv == 0:
            mean1 = np.nan
            mean2 = np.sqrt(((refs_all[:, None, :] - proj[None, :, :]) ** 2)
                            .sum(-1)).min(1).mean() if len(proj) else np.nan
            out = np.float32(0.5 * (mean1 + mean2))
        else:
            d2 = ((pv[:, None, :] - refs_all[None, :, :]) ** 2).sum(-1)
            mean1 = np.sqrt(d2.min(1)).mean()
            mean2 = np.sqrt(d2.min(0)).mean()
            out = np.float32(0.5 * (mean1 + mean2))
        if _trace:
            return out, None
        return out

    pvs = pv[np.argsort(pv[:, 0], kind="stable")] - c
    px = pvs[:, 0]
    py_lo, py_hi = pvs[:, 1].min(), pvs[:, 1].max()
    rsx = refs_all[np.argsort(refs_all[:, 0], kind="stable")] - c
    rx = rsx[:, 0]

    R_pc = int(np.ceil(nv / (NCORES * 128))) * 128
    NP = NCORES * R_pc
    T_p = R_pc // 128
    T_tot = NP // 128

    # ---- proj-side pass: rows = sorted valid proj, candidates = refs ----
    A = np.concatenate([_enc_a(pvs), np.zeros((K, NP - nv), _f16)], axis=1)
    A[8, nv:] = _f16(BIGVAL / P2SCALE)
    B = _enc_b(rsx)

    p_offs = np.zeros(T_tot, np.int64)
    for g in range(T_tot):
        lo, hi = 128 * g, min(128 * (g + 1), nv)
        if lo >= nv:
            continue
        p_offs[g] = _window_offsets(px[lo], px[hi - 1], rx, M, CP)
    bgath = _gather_windows(B, M, p_offs, CP)

    # ---- ref-side pass: rows = refs (class-ordered), candidates = proj ----
    far_top = rsx[:, 1] > py_hi + TAU
    far_bot = rsx[:, 1] < py_lo - TAU
    near_i = np.where(~(far_top | far_bot))[0]
    n_keep = (len(near_i) // 128) * 128
    if n_keep < len(near_i):
        by_y = near_i[np.argsort(np.abs(rsx[near_i][:, 1]), kind="stable")]
        keep, movers = by_y[:n_keep], by_y[n_keep:]
    else:
        keep, movers = near_i, np.array([], np.int64)
    top_i = np.concatenate([np.where(far_top)[0], movers]).astype(np.int64)
    bot_i = np.where(far_bot)[0]
    ordr = np.concatenate([
        keep[np.argsort(rsx[keep][:, 0], kind="stable")],
        top_i[np.argsort(rsx[top_i][:, 0], kind="stable")],
        bot_i[np.argsort(rsx[bot_i][:, 0], kind="stable")],
    ])
    rs2 = rsx[ordr]
    n_near, n_top = len(keep), len(top_i)

    band_t = np.where(pvs[:, 1] >= py_hi - BAND_W)[0]
    band_b = np.where(pvs[:, 1] <= py_lo + BAND_W)[0]
    btx = pvs[band_t][:, 0]
    bbx = pvs[band_b][:, 0]
    BT = _enc_b(pvs[band_t]) if len(band_t) else np.zeros((K, 0), _f16)
    BB = _enc_b(pvs[band_b]) if len(band_b) else np.zeros((K, 0), _f16)

    AT = _enc_a(rs2)
    r_offs = np.zeros(M // 128, np.int64)
    r_kind = [None] * (M // 128)
    for u in range(M // 128):
        lo, hi = 128 * u, 128 * (u + 1)
        if hi <= n_near:
            kind = "near"
        elif lo >= n_near and hi <= n_near + n_top:
            kind = "top"
        elif lo >= n_near + n_top:
            kind = "bot"
        else:
            kind = "top" if (hi - n_near) > 64 and len(btx) else "near"
            if lo >= n_near and len(bbx) and (hi - (n_near + n_top)) > 64:
                kind = "bot"
        r_kind[u] = kind
        cx = {"near": px, "top": btx, "bot": bbx}[kind]
        xlo, xhi = rs2[lo:hi, 0].min(), rs2[lo:hi, 0].max()
        r_offs[u] = _window_offsets(xlo, xhi, cx, len(cx), CR)
    BPm = _enc_b(pvs)
    src = {"near": (BPm, nv), "top": (BT, len(btx)), "bot": (BB, len(bbx))}
    bpg = np.concatenate([
        _gather_windows(src[r_kind[u]][0], src[r_kind[u]][1], [r_offs[u]], CR)
        for u in range(M // 128)
    ], axis=1)

    # ---- run on hardware ----
    in_maps = []
    for cc in range(NCORES):
        ab = np.concatenate([
            A[:, cc * R_pc:(cc + 1) * R_pc],
            bgath[:, cc * T_p * CP:(cc + 1) * T_p * CP],
        ], axis=1)
        cd = np.concatenate([
            AT[:, cc * REF_TILES * 128:(cc + 1) * REF_TILES * 128],
            bpg[:, cc * REF_TILES * CR:(cc + 1) * REF_TILES * CR],
        ], axis=1)
        in_maps.append({"ab_in": np.ascontiguousarray(ab),
                        "cd_in": np.ascontiguousarray(cd)})

    res = _run_on_hw(in_maps, T_p, trace=_trace, tmpdir=_tmpdir)

    rowd2 = np.empty(NP)
    refd2 = np.empty(M)
    for cc in range(NCORES):
        out = res.results[cc]
        rowd2[cc * R_pc:(cc + 1) * R_pc] = \
            out["rowm_out"].astype(np.float64).T.reshape(-1)
        refd2[cc * REF_TILES * 128:(cc + 1) * REF_TILES * 128] = \
            out["refm_out"].astype(np.float64).T.reshape(-1)

    # ---- host: margins, fallback, means ----
    ry_lo, ry_hi = rsx[:, 1].min(), rsx[:, 1].max()
    found1 = np.sqrt(np.maximum(rowd2[:nv], 0.0))
    yc1 = np.maximum(0.0, np.maximum(pvs[:, 1] - ry_hi, ry_lo - pvs[:, 1]))
    marg1 = np.full(nv, np.inf)
    for g in range((nv + 127) // 128):
        lo, hi = 128 * g, min(128 * (g + 1), nv)
        marg1[lo:hi] = _edge_margins(px[lo:hi], yc1[lo:hi], rx, M,
                                     int(p_offs[g]), CP)
    slack1 = np.maximum(1e-3 * found1, 1e-4)
    bad1 = (found1 > marg1 - slack1) | ~np.isfinite(found1)
    if bad1.any():
        ii = np.where(bad1)[0]
        d2x = ((pvs[ii, None, :] - rsx[None, :, :]) ** 2).sum(-1).min(1)
        found1[ii] = np.sqrt(d2x)
    mean1 = found1.mean()

    found2 = np.sqrt(np.maximum(refd2, 0.0))
    yc2 = np.maximum(0.0, np.maximum(rs2[:, 1] - py_hi, py_lo - rs2[:, 1]))
    marg2 = np.full(M, np.inf)
    for u in range(M // 128):
        lo, hi = 128 * u, 128 * (u + 1)
        kind = r_kind[u]
        cx = {"near": px, "top": btx, "bot": bbx}[kind]
        m = _edge_margins(rs2[lo:hi, 0], yc2[lo:hi], cx, len(cx),
                          int(r_offs[u]), CR)
        if kind == "top":
            m = np.minimum(np.maximum(rs2[lo:hi, 1] - (py_hi - BAND_W), 0.0),
                           m)
        elif kind == "bot":
            m = np.minimum(np.maximum((py_lo + BAND_W) - rs2[lo:hi, 1], 0.0),
                           m)
        marg2[lo:hi] = m
    slack2 = np.maximum(1e-3 * found2, 1e-4)
    bad2 = (found2 > marg2 - slack2) | ~np.isfinite(found2)
    if bad2.any():
        jj = np.where(bad2)[0]
        d2x = ((rs2[jj, None, :] - pvs[None, :, :]) ** 2).sum(-1).min(1)
        found2[jj] = np.sqrt(d2x)
    mean2 = found2.mean()

    out = np.float32(0.5 * (mean1 + mean2))
    if _trace:
        return out, res
    return out


# revision 28
# speedup vs baseline: 8.8588x; 1.0088x over previous
"""CenterlineLoss Trainium2 kernel — windowed two-pass nearest-neighbor.

Computes 0.5*(mean1 + mean2) where
  mean1 = mean over valid proj points of distance to nearest ref point
  mean2 = mean over ref points of distance to nearest valid proj point
(reference semantics: ref coords swapped; proj row order irrelevant;
proj validity mask applied to both reductions).

Strategy: the all-pairs [N, M] distance matrix is never materialized.
Host sorts the valid proj points and the refs along x and gathers, for
every 128-point tile, a contiguous candidate window (384 refs per proj
tile, 512 proj per ref tile) from the other (sorted) point set.  The
device computes, per tile, a [128, w] squared-distance block via one
TensorE matmul (K=14 fp16 limb-split encoding, d^2 exact to fp32);
tiles are grouped four-to-a-PSUM-allocation and retired by a single
strided DVE tensor_reduce into per-row minima.  Refs whose y lies
beyond the proj y-extent get their candidates from a boundary band of
proj sorted by x instead.

Correctness does not depend on the windows: the host computes, per
query row, a lower bound on the distance to any EXCLUDED candidate
(x-gap to the window edge, y-clearance to the set extent, band bound).
Rows whose found min does not beat that bound are recomputed exactly
on the host (typically 0-2 rows).  Degenerate inputs (few valid
points) fall back to an exact host computation.
"""

import time

import numpy as np

import concourse.bacc as bacc
import concourse.mybir as mybir
import concourse.tile as tile
from concourse import bass_utils

N = 16384
M = 8192
NCORES = 8
K = 14                      # limb-split contraction depth
CP = 384                    # proj-side candidate window per 128-row tile
CR = 512                    # ref-side candidate window per 128-row tile
P2SCALE = 64.0
R2SCALE = 16.0
BIGVAL = 60000.0            # sentinel d^2 (> any real window min)
CENTER = (320.0, 240.0)
BAND_W = 48.0               # boundary-band depth for far refs
TAU = 2.0                   # y-clearance above which a ref is "far"
REF_TILES = M // 128 // NCORES  # 8 ref tiles per core

_f16 = np.float16


def _split2(v):
    h = v.astype(_f16).astype(np.float64)
    l = (v - h).astype(_f16).astype(np.float64)
    return h, l


def _split3(v):
    h = v.astype(_f16).astype(np.float64)
    r = v - h
    m = r.astype(_f16).astype(np.float64)
    l = (r - m).astype(_f16).astype(np.float64)
    return h, m, l


def _enc_a(pts):
    """Row-side limb encoding (points on the partition axis). [n,2]->[K,n]"""
    x = pts[:, 0]
    y = pts[:, 1]
    Xh, Xl = _split2(x)
    Yh, Yl = _split2(y)
    px, py = Xh + Xl, Yh + Yl
    P2a, P2b, P2c = _split3((px * px + py * py) / P2SCALE)
    rs = np.full(len(x), R2SCALE)
    return np.stack(
        [Xh, Xh, Xl, Xl, Yh, Yh, Yl, Yl, P2a, P2b, P2c, rs, rs, rs]
    ).astype(_f16)


def _enc_b(pts):
    """Column-side limb encoding (candidate points). [n,2]->[K,n]"""
    x = pts[:, 0]
    y = pts[:, 1]
    Xh, Xl = _split2(x)
    Yh, Yl = _split2(y)
    rx, ry = Xh + Xl, Yh + Yl
    R2a, R2b, R2c = _split3((rx * rx + ry * ry) / R2SCALE)
    ps = np.full(len(x), P2SCALE)
    return np.stack(
        [-2 * Xh, -2 * Xl, -2 * Xh, -2 * Xl,
         -2 * Yh, -2 * Yl, -2 * Yh, -2 * Yl,
         ps, ps, ps, R2a, R2b, R2c]
    ).astype(_f16)


_B_SENT = None


def _b_sentinel():
    """Candidate-side sentinel column: d^2 == BIGVAL against any row."""
    global _B_SENT
    if _B_SENT is None:
        col = np.zeros((K, 1), _f16)
        col[11, 0] = _f16(BIGVAL / R2SCALE)
        _B_SENT = col
    return _B_SENT


_PROGRAM_CACHE = {}

# NOTE on rejected variants (hardware constraints discovered on the way):
# GPSIMD cannot run TensorTensor or touch PSUM; DVE may read at most one
# PSUM operand per instruction; TENSOR_TENSOR_REDUCE with op=min crashed
# the exec unit.  A plain strided tensor_reduce from PSUM is both legal
# and the fastest schedule found.


def _build_program(T_p=14):
    key = T_p
    if key in _PROGRAM_CACHE:
        return _PROGRAM_CACHE[key]

    f16 = mybir.dt.float16
    f32 = mybir.dt.float32
    MIN = mybir.AluOpType.min

    nc = bacc.Bacc("TRN2", target_bir_lowering=False, debug=False,
                   num_devices=NCORES)

    WAB = T_p * (128 + CP)
    WCD = REF_TILES * (128 + CR)
    ab_dram = nc.dram_tensor("ab_in", [K, WAB], f16, kind="ExternalInput").ap()
    cd_dram = nc.dram_tensor("cd_in", [K, WCD], f16, kind="ExternalInput").ap()
    rowm_dram = nc.dram_tensor("rowm_out", [128, T_p], f32,
                               kind="ExternalOutput").ap()
    refm_dram = nc.dram_tensor("refm_out", [128, REF_TILES], f32,
                               kind="ExternalOutput").ap()

    with tile.TileContext(nc) as tc, \
            tc.tile_pool(name="const", bufs=1) as cpool:
        ab_sb = cpool.tile([K, WAB], f16, tag="ab")
        cd_sb = cpool.tile([K, WCD], f16, tag="cd")
        rowm = cpool.tile([128, T_p], f32, tag="rowm")
        refm = cpool.tile([128, REF_TILES], f32, tag="refm")

        # inputs on two separate queues so both are in flight immediately
        nc.sync.dma_start(ab_sb[:], ab_dram)
        nc.scalar.dma_start(cd_sb[:], cd_dram)

        # matmul tiles are grouped 4-per-PSUM-allocation (at 512-col
        # stride = one bank per tile) so a single strided tensor_reduce
        # [128, g, w] -> [128, g] retires a whole group.  The expensive
        # ref groups run in the middle; a cheap proj group retires last
        # so the final output DMA tails a short op.
        with tc.tile_pool(name="mm", bufs=2, space="PSUM") as pspool:

            def do_group(tiles, a_base, b_base, w, acc, res_sb):
                g = len(tiles)
                ps = pspool.tile([128, 2048], f32, tag="mm")
                for k, t in enumerate(tiles):
                    nc.tensor.matmul(
                        ps[:, k * 512:k * 512 + w],
                        res_sb[:, a_base + t * 128:a_base + (t + 1) * 128],
                        res_sb[:, b_base + t * w:b_base + (t + 1) * w],
                        start=True, stop=True)
                view = ps[:].rearrange("p (b f) -> p b f", f=512)
                nc.vector.tensor_reduce(acc, view[:, :g, :w], op=MIN,
                                        axis=mybir.AxisListType.X)

            pgroups = [list(range(t0, min(t0 + 4, T_p)))
                       for t0 in range(0, T_p, 4)]
            rgroups = [list(range(u0, min(u0 + 4, REF_TILES)))
                       for u0 in range(0, REF_TILES, 4)]
            for kind, tiles in ([("p", t) for t in pgroups[:2]]
                                + [("r", t) for t in rgroups]
                                + [("p", t) for t in pgroups[2:]]):
                if kind == "p":
                    do_group(tiles, 0, T_p * 128, CP,
                             rowm[:, tiles[0]:tiles[0] + len(tiles)], ab_sb)
                else:
                    do_group(tiles, 0, REF_TILES * 128, CR,
                             refm[:, tiles[0]:tiles[0] + len(tiles)], cd_sb)
                    if tiles[0] + len(tiles) == REF_TILES:
                        nc.sync.dma_start(refm_dram, refm[:])
            nc.sync.dma_start(rowm_dram, rowm[:])

    nc.compile()
    _PROGRAM_CACHE[key] = nc
    return nc


def _gather_windows(enc, n_real, offs, w):
    """Stack enc[:, o:o+w] slices; pad short sources with sentinels."""
    cols = []
    for o in offs:
        if n_real >= w:
            cols.append(enc[:, o:o + w])
        else:
            pad = np.broadcast_to(_b_sentinel(), (K, w - n_real))
            cols.append(np.concatenate([enc[:, :n_real], pad], axis=1))
    return np.concatenate(cols, axis=1)


def _window_offsets(tile_lo_x, tile_hi_x, cand_x, n_cand, w):
    ja = np.searchsorted(cand_x, tile_lo_x)
    jb = np.searchsorted(cand_x, tile_hi_x)
    return int(np.clip((ja + jb) // 2 - w // 2, 0, max(0, n_cand - w)))


def _edge_margins(qx, yclear, cand_x, n_cand, o, w):
    """Min distance from query rows to any candidate excluded by the
    x-window [o, o+w) — hypot of x-gap past the nearest excluded
    element and the y-clearance to the candidate set's y-extent."""
    n = len(qx)
    if o > 0:
        ml = np.hypot(np.maximum(qx - cand_x[o - 1], 0.0), yclear)
    else:
        ml = np.full(n, np.inf)
    if o + w < n_cand:
        mr = np.hypot(np.maximum(cand_x[o + w] - qx, 0.0), yclear)
    else:
        mr = np.full(n, np.inf)
    return np.minimum(ml, mr)


def _run_on_hw(in_maps, T_p, trace=False, tmpdir=None):
    nc = _build_program(T_p)
    last = None
    for wait_s in (0, 30, 60, 90):
        if wait_s:
            time.sleep(wait_s)
        try:
            return bass_utils.run_bass_kernel_spmd(
                nc, in_maps, core_ids=list(range(NCORES)), trace=trace,
                tmpdir=tmpdir,
            )
        except Exception as e:
            last = e
    raise last


def kernel(bezier_proj_centerline_img, ref_catheter_centerline, _trace=False,
           _tmpdir=None):
    proj = np.asarray(bezier_proj_centerline_img, np.float64)
    refs_all = np.asarray(ref_catheter_centerline, np.float64)[:, ::-1]
    c = np.array(CENTER)

    mask = (
        (proj[:, 0] >= 0.0) & (proj[:, 0] <= 640.0)
        & (proj[:, 1] >= 0.0) & (proj[:, 1] <= 480.0)
    )
    pv = proj[mask]
    nv = len(pv)
    m_ref = len(refs_all)

    if nv < 2 * CP or m_ref != M:
        # degenerate input: exact host computation
        if nv == 0:
            mean1 = np.nan
            mean2 = np.sqrt(((refs_all[:, None, :] - proj[None, :, :]) ** 2)
                            .sum(-1)).min(1).mean() if len(proj) else np.nan
            out = np.float32(0.5 * (mean1 + mean2))
        else:
            d2 = ((pv[:, None, :] - refs_all[None, :, :]) ** 2).sum(-1)
            mean1 = np.sqrt(d2.min(1)).mean()
            mean2 = np.sqrt(d2.min(0)).mean()
            out = np.float32(0.5 * (mean1 + mean2))
        if _trace:
            return out, None
        return out

    pvs = pv[np.argsort(pv[:, 0], kind="stable")] - c
    px = pvs[:, 0]
    py_lo, py_hi = pvs[:, 1].min(), pvs[:, 1].max()
    rsx = refs_all[np.argsort(refs_all[:, 0], kind="stable")] - c
    rx = rsx[:, 0]

    R_pc = int(np.ceil(nv / (NCORES * 128))) * 128
    NP = NCORES * R_pc
    T_p = R_pc // 128
    T_tot = NP // 128

    # ---- proj-side pass: rows = sorted valid proj, candidates = refs ----
    A = np.concatenate([_enc_a(pvs), np.zeros((K, NP - nv), _f16)], axis=1)
    A[8, nv:] = _f16(BIGVAL / P2SCALE)
    B = _enc_b(rsx)

    p_offs = np.zeros(T_tot, np.int64)
    for g in range(T_tot):
        lo, hi = 128 * g, min(128 * (g + 1), nv)
        if lo >= nv:
            continue
        p_offs[g] = _window_offsets(px[lo], px[hi - 1], rx, M, CP)
    bgath = _gather_windows(B, M, p_offs, CP)

    # ---- ref-side pass: rows = refs (class-ordered), candidates = proj ----
    far_top = rsx[:, 1] > py_hi + TAU
    far_bot = rsx[:, 1] < py_lo - TAU
    near_i = np.where(~(far_top | far_bot))[0]
    n_keep = (len(near_i) // 128) * 128
    if n_keep < len(near_i):
        by_y = near_i[np.argsort(np.abs(rsx[near_i][:, 1]), kind="stable")]
        keep, movers = by_y[:n_keep], by_y[n_keep:]
    else:
        keep, movers = near_i, np.array([], np.int64)
    top_i = np.concatenate([np.where(far_top)[0], movers]).astype(np.int64)
    bot_i = np.where(far_bot)[0]
    ordr = np.concatenate([
        keep[np.argsort(rsx[keep][:, 0], kind="stable")],
        top_i[np.argsort(rsx[top_i][:, 0], kind="stable")],
        bot_i[np.argsort(rsx[bot_i][:, 0], kind="stable")],
    ])
    rs2 = rsx[ordr]
    n_near, n_top = len(keep), len(top_i)

    band_t = np.where(pvs[:, 1] >= py_hi - BAND_W)[0]
    band_b = np.where(pvs[:, 1] <= py_lo + BAND_W)[0]
    btx = pvs[band_t][:, 0]
    bbx = pvs[band_b][:, 0]
    BT = _enc_b(pvs[band_t]) if len(band_t) else np.zeros((K, 0), _f16)
    BB = _enc_b(pvs[band_b]) if len(band_b) else np.zeros((K, 0), _f16)

    AT = _enc_a(rs2)
    r_offs = np.zeros(M // 128, np.int64)
    r_kind = [None] * (M // 128)
    for u in range(M // 128):
        lo, hi = 128 * u, 128 * (u + 1)
        if hi <= n_near:
            kind = "near"
        elif lo >= n_near and hi <= n_near + n_top:
            kind = "top"
        elif lo >= n_near + n_top:
            kind = "bot"
        else:
            kind = "top" if (hi - n_near) > 64 and len(btx) else "near"
            if lo >= n_near and len(bbx) and (hi - (n_near + n_top)) > 64:
                kind = "bot"
        r_kind[u] = kind
        cx = {"near": px, "top": btx, "bot": bbx}[kind]
        xlo, xhi = rs2[lo:hi, 0].min(), rs2[lo:hi, 0].max()
        r_offs[u] = _window_offsets(xlo, xhi, cx, len(cx), CR)
    BPm = _enc_b(pvs)
    src = {"near": (BPm, nv), "top": (BT, len(btx)), "bot": (BB, len(bbx))}
    bpg = np.concatenate([
        _gather_windows(src[r_kind[u]][0], src[r_kind[u]][1], [r_offs[u]], CR)
        for u in range(M // 128)
    ], axis=1)

    # ---- run on hardware ----
    in_maps = []
    for cc in range(NCORES):
        ab = np.concatenate([
            A[:, cc * R_pc:(cc + 1) * R_pc],
            bgath[:, cc * T_p * CP:(cc + 1) * T_p * CP],
        ], axis=1)
        cd = np.concatenate([
            AT[:, cc * REF_TILES * 128:(cc + 1) * REF_TILES * 128],
            bpg[:, cc * REF_TILES * CR:(cc + 1) * REF_TILES * CR],
        ], axis=1)
        in_maps.append({"ab_in": np.ascontiguousarray(ab),
                        "cd_in": np.ascontiguousarray(cd)})

    res = _run_on_hw(in_maps, T_p, trace=_trace, tmpdir=_tmpdir)

    rowd2 = np.empty(NP)
    refd2 = np.empty(M)
    for cc in range(NCORES):
        out = res.results[cc]
        rowd2[cc * R_pc:(cc + 1) * R_pc] = \
            out["rowm_out"].astype(np.float64).T.reshape(-1)
        refd2[cc * REF_TILES * 128:(cc + 1) * REF_TILES * 128] = \
            out["refm_out"].astype(np.float64).T.reshape(-1)

    # ---- host: margins, fallback, means ----
    ry_lo, ry_hi = rsx[:, 1].min(), rsx[:, 1].max()
    found1 = np.sqrt(np.maximum(rowd2[:nv], 0.0))
    yc1 = np.maximum(0.0, np.maximum(pvs[:, 1] - ry_hi, ry_lo - pvs[:, 1]))
    marg1 = np.full(nv, np.inf)
    for g in range((nv + 127) // 128):
        lo, hi = 128 * g, min(128 * (g + 1), nv)
        marg1[lo:hi] = _edge_margins(px[lo:hi], yc1[lo:hi], rx, M,
                                     int(p_offs[g]), CP)
    slack1 = np.maximum(1e-3 * found1, 1e-4)
    bad1 = (found1 > marg1 - slack1) | ~np.isfinite(found1)
    if bad1.any():
        ii = np.where(bad1)[0]
        d2x = ((pvs[ii, None, :] - rsx[None, :, :]) ** 2).sum(-1).min(1)
        found1[ii] = np.sqrt(d2x)
    mean1 = found1.mean()

    found2 = np.sqrt(np.maximum(refd2, 0.0))
    yc2 = np.maximum(0.0, np.maximum(rs2[:, 1] - py_hi, py_lo - rs2[:, 1]))
    marg2 = np.full(M, np.inf)
    for u in range(M // 128):
        lo, hi = 128 * u, 128 * (u + 1)
        kind = r_kind[u]
        cx = {"near": px, "top": btx, "bot": bbx}[kind]
        m = _edge_margins(rs2[lo:hi, 0], yc2[lo:hi], cx, len(cx),
                          int(r_offs[u]), CR)
        if kind == "top":
            m = np.minimum(np.maximum(rs2[lo:hi, 1] - (py_hi - BAND_W), 0.0),
                           m)
        elif kind == "bot":
            m = np.minimum(np.maximum((py_lo + BAND_W) - rs2[lo:hi, 1], 0.0),
                           m)
        marg2[lo:hi] = m
    slack2 = np.maximum(1e-3 * found2, 1e-4)
    bad2 = (found2 > marg2 - slack2) | ~np.isfinite(found2)
    if bad2.any():
        jj = np.where(bad2)[0]
        d2x = ((rs2[jj, None, :] - pvs[None, :, :]) ** 2).sum(-1).min(1)
        found2[jj] = np.sqrt(d2x)
    mean2 = found2.mean()

    out = np.float32(0.5 * (mean1 + mean2))
    if _trace:
        return out, res
    return out


# revision 33
# speedup vs baseline: 9.4352x; 1.0651x over previous
"""CenterlineLoss Trainium2 kernel — windowed two-pass nearest-neighbor.

Computes 0.5*(mean1 + mean2) where
  mean1 = mean over valid proj points of distance to nearest ref point
  mean2 = mean over ref points of distance to nearest valid proj point
(reference semantics: ref coords swapped; proj row order irrelevant;
proj validity mask applied to both reductions).

Strategy: the all-pairs [N, M] distance matrix is never materialized.
Host sorts the valid proj points and the refs along x and gathers, for
every 128-point tile, a contiguous candidate window (384 refs per proj
tile, 512 proj per ref tile) from the other (sorted) point set.  The
device computes, per tile, a [128, w] squared-distance block via one
TensorE matmul (K=14 fp16 limb-split encoding, d^2 exact to fp32);
tiles are grouped four-to-a-PSUM-allocation and retired by a single
strided DVE tensor_reduce into per-row minima.  Refs whose y lies
beyond the proj y-extent get their candidates from a boundary band of
proj sorted by x instead.

Correctness does not depend on the windows: the host computes, per
query row, a lower bound on the distance to any EXCLUDED candidate
(x-gap to the window edge, y-clearance to the set extent, band bound).
Rows whose found min does not beat that bound are recomputed exactly
on the host (typically 0-2 rows).  Degenerate inputs (few valid
points) fall back to an exact host computation.
"""

import time

import numpy as np

import concourse.bacc as bacc
import concourse.mybir as mybir
import concourse.tile as tile
from concourse import bass_utils

N = 16384
M = 8192
NCORES = 8
K = 14                      # limb-split contraction depth
CP = 352                    # proj-side candidate window per 128-row tile
CR = 448                    # ref-side candidate window per 128-row tile
P2SCALE = 64.0
R2SCALE = 16.0
BIGVAL = 60000.0            # sentinel d^2 (> any real window min)
CENTER = (320.0, 240.0)
BAND_W = 48.0               # boundary-band depth for far refs
TAU = 2.0                   # y-clearance above which a ref is "far"
REF_TILES = M // 128 // NCORES  # 8 ref tiles per core

_f16 = np.float16


def _split2(v):
    h = v.astype(_f16).astype(np.float64)
    l = (v - h).astype(_f16).astype(np.float64)
    return h, l


def _split3(v):
    h = v.astype(_f16).astype(np.float64)
    r = v - h
    m = r.astype(_f16).astype(np.float64)
    l = (r - m).astype(_f16).astype(np.float64)
    return h, m, l


def _enc_a(pts):
    """Row-side limb encoding (points on the partition axis). [n,2]->[K,n]"""
    x = pts[:, 0]
    y = pts[:, 1]
    Xh, Xl = _split2(x)
    Yh, Yl = _split2(y)
    px, py = Xh + Xl, Yh + Yl
    P2a, P2b, P2c = _split3((px * px + py * py) / P2SCALE)
    rs = np.full(len(x), R2SCALE)
    return np.stack(
        [Xh, Xh, Xl, Xl, Yh, Yh, Yl, Yl, P2a, P2b, P2c, rs, rs, rs]
    ).astype(_f16)


def _enc_b(pts):
    """Column-side limb encoding (candidate points). [n,2]->[K,n]"""
    x = pts[:, 0]
    y = pts[:, 1]
    Xh, Xl = _split2(x)
    Yh, Yl = _split2(y)
    rx, ry = Xh + Xl, Yh + Yl
    R2a, R2b, R2c = _split3((rx * rx + ry * ry) / R2SCALE)
    ps = np.full(len(x), P2SCALE)
    return np.stack(
        [-2 * Xh, -2 * Xl, -2 * Xh, -2 * Xl,
         -2 * Yh, -2 * Yl, -2 * Yh, -2 * Yl,
         ps, ps, ps, R2a, R2b, R2c]
    ).astype(_f16)


_B_SENT = None


def _b_sentinel():
    """Candidate-side sentinel column: d^2 == BIGVAL against any row."""
    global _B_SENT
    if _B_SENT is None:
        col = np.zeros((K, 1), _f16)
        col[11, 0] = _f16(BIGVAL / R2SCALE)
        _B_SENT = col
    return _B_SENT


_PROGRAM_CACHE = {}

# NOTE on rejected variants (hardware constraints discovered on the way):
# GPSIMD cannot run TensorTensor or touch PSUM; DVE may read at most one
# PSUM operand per instruction; TENSOR_TENSOR_REDUCE with op=min crashed
# the exec unit.  A plain strided tensor_reduce from PSUM is both legal
# and the fastest schedule found.


def _build_program(T_p=14):
    key = T_p
    if key in _PROGRAM_CACHE:
        return _PROGRAM_CACHE[key]

    f16 = mybir.dt.float16
    f32 = mybir.dt.float32
    MIN = mybir.AluOpType.min

    nc = bacc.Bacc("TRN2", target_bir_lowering=False, debug=False,
                   num_devices=NCORES)

    WAB = T_p * (128 + CP)
    WCD = REF_TILES * (128 + CR)
    ab_dram = nc.dram_tensor("ab_in", [K, WAB], f16, kind="ExternalInput").ap()
    cd_dram = nc.dram_tensor("cd_in", [K, WCD], f16, kind="ExternalInput").ap()
    rowm_dram = nc.dram_tensor("rowm_out", [128, T_p], f32,
                               kind="ExternalOutput").ap()
    refm_dram = nc.dram_tensor("refm_out", [128, REF_TILES], f32,
                               kind="ExternalOutput").ap()

    with tile.TileContext(nc) as tc, \
            tc.tile_pool(name="const", bufs=1) as cpool:
        ab_sb = cpool.tile([K, WAB], f16, tag="ab")
        cd_sb = cpool.tile([K, WCD], f16, tag="cd")
        rowm = cpool.tile([128, T_p], f32, tag="rowm")
        refm = cpool.tile([128, REF_TILES], f32, tag="refm")

        # inputs on two separate queues so both are in flight immediately
        nc.sync.dma_start(ab_sb[:], ab_dram)
        nc.scalar.dma_start(cd_sb[:], cd_dram)

        # matmul tiles are grouped 4-per-PSUM-allocation (at 512-col
        # stride = one bank per tile) so a single strided tensor_reduce
        # [128, g, w] -> [128, g] retires a whole group.  The expensive
        # ref groups run in the middle; a cheap proj group retires last
        # so the final output DMA tails a short op.
        with tc.tile_pool(name="mm", bufs=2, space="PSUM") as pspool:

            def do_group(tiles, a_base, b_base, w, acc, res_sb):
                g = len(tiles)
                ps = pspool.tile([128, 2048], f32, tag="mm")
                for k, t in enumerate(tiles):
                    nc.tensor.matmul(
                        ps[:, k * 512:k * 512 + w],
                        res_sb[:, a_base + t * 128:a_base + (t + 1) * 128],
                        res_sb[:, b_base + t * w:b_base + (t + 1) * w],
                        start=True, stop=True)
                view = ps[:].rearrange("p (b f) -> p b f", f=512)
                nc.vector.tensor_reduce(acc, view[:, :g, :w], op=MIN,
                                        axis=mybir.AxisListType.X)

            pgroups = [list(range(t0, min(t0 + 4, T_p)))
                       for t0 in range(0, T_p, 4)]
            rgroups = [list(range(u0, min(u0 + 4, REF_TILES)))
                       for u0 in range(0, REF_TILES, 4)]
            for kind, tiles in ([("p", t) for t in pgroups[:2]]
                                + [("r", t) for t in rgroups]
                                + [("p", t) for t in pgroups[2:]]):
                if kind == "p":
                    do_group(tiles, 0, T_p * 128, CP,
                             rowm[:, tiles[0]:tiles[0] + len(tiles)], ab_sb)
                else:
                    do_group(tiles, 0, REF_TILES * 128, CR,
                             refm[:, tiles[0]:tiles[0] + len(tiles)], cd_sb)
                    if tiles[0] + len(tiles) == REF_TILES:
                        nc.sync.dma_start(refm_dram, refm[:])
            nc.sync.dma_start(rowm_dram, rowm[:])

    nc.compile()
    _PROGRAM_CACHE[key] = nc
    return nc


def _gather_windows(enc, n_real, offs, w):
    """Stack enc[:, o:o+w] slices; pad short sources with sentinels."""
    cols = []
    for o in offs:
        if n_real >= w:
            cols.append(enc[:, o:o + w])
        else:
            pad = np.broadcast_to(_b_sentinel(), (K, w - n_real))
            cols.append(np.concatenate([enc[:, :n_real], pad], axis=1))
    return np.concatenate(cols, axis=1)


def _window_offsets(tile_lo_x, tile_hi_x, cand_x, n_cand, w):
    ja = np.searchsorted(cand_x, tile_lo_x)
    jb = np.searchsorted(cand_x, tile_hi_x)
    return int(np.clip((ja + jb) // 2 - w // 2, 0, max(0, n_cand - w)))


def _edge_margins(qx, yclear, cand_x, n_cand, o, w):
    """Min distance from query rows to any candidate excluded by the
    x-window [o, o+w) — hypot of x-gap past the nearest excluded
    element and the y-clearance to the candidate set's y-extent."""
    n = len(qx)
    if o > 0:
        ml = np.hypot(np.maximum(qx - cand_x[o - 1], 0.0), yclear)
    else:
        ml = np.full(n, np.inf)
    if o + w < n_cand:
        mr = np.hypot(np.maximum(cand_x[o + w] - qx, 0.0), yclear)
    else:
        mr = np.full(n, np.inf)
    return np.minimum(ml, mr)


def _run_on_hw(in_maps, T_p, trace=False, tmpdir=None):
    nc = _build_program(T_p)
    last = None
    for wait_s in (0, 30, 60, 90):
        if wait_s:
            time.sleep(wait_s)
        try:
            return bass_utils.run_bass_kernel_spmd(
                nc, in_maps, core_ids=list(range(NCORES)), trace=trace,
                tmpdir=tmpdir,
            )
        except Exception as e:
            last = e
    raise last


def kernel(bezier_proj_centerline_img, ref_catheter_centerline, _trace=False,
           _tmpdir=None):
    proj = np.asarray(bezier_proj_centerline_img, np.float64)
    refs_all = np.asarray(ref_catheter_centerline, np.float64)[:, ::-1]
    c = np.array(CENTER)

    mask = (
        (proj[:, 0] >= 0.0) & (proj[:, 0] <= 640.0)
        & (proj[:, 1] >= 0.0) & (proj[:, 1] <= 480.0)
    )
    pv = proj[mask]
    nv = len(pv)
    m_ref = len(refs_all)

    if nv < 2 * CP or m_ref != M:
        # degenerate input: exact host computation
        if nv == 0:
            mean1 = np.nan
            mean2 = np.sqrt(((refs_all[:, None, :] - proj[None, :, :]) ** 2)
                            .sum(-1)).min(1).mean() if len(proj) else np.nan
            out = np.float32(0.5 * (mean1 + mean2))
        else:
            d2 = ((pv[:, None, :] - refs_all[None, :, :]) ** 2).sum(-1)
            mean1 = np.sqrt(d2.min(1)).mean()
            mean2 = np.sqrt(d2.min(0)).mean()
            out = np.float32(0.5 * (mean1 + mean2))
        if _trace:
            return out, None
        return out

    pvs = pv[np.argsort(pv[:, 0], kind="stable")] - c
    px = pvs[:, 0]
    py_lo, py_hi = pvs[:, 1].min(), pvs[:, 1].max()
    rsx = refs_all[np.argsort(refs_all[:, 0], kind="stable")] - c
    rx = rsx[:, 0]

    R_pc = int(np.ceil(nv / (NCORES * 128))) * 128
    NP = NCORES * R_pc
    T_p = R_pc // 128
    T_tot = NP // 128

    # ---- proj-side pass: rows = sorted valid proj, candidates = refs ----
    A = np.concatenate([_enc_a(pvs), np.zeros((K, NP - nv), _f16)], axis=1)
    A[8, nv:] = _f16(BIGVAL / P2SCALE)
    B = _enc_b(rsx)

    p_offs = np.zeros(T_tot, np.int64)
    for g in range(T_tot):
        lo, hi = 128 * g, min(128 * (g + 1), nv)
        if lo >= nv:
            continue
        p_offs[g] = _window_offsets(px[lo], px[hi - 1], rx, M, CP)
    bgath = _gather_windows(B, M, p_offs, CP)

    # ---- ref-side pass: rows = refs (class-ordered), candidates = proj ----
    far_top = rsx[:, 1] > py_hi + TAU
    far_bot = rsx[:, 1] < py_lo - TAU
    near_i = np.where(~(far_top | far_bot))[0]
    n_keep = (len(near_i) // 128) * 128
    if n_keep < len(near_i):
        by_y = near_i[np.argsort(np.abs(rsx[near_i][:, 1]), kind="stable")]
        keep, movers = by_y[:n_keep], by_y[n_keep:]
    else:
        keep, movers = near_i, np.array([], np.int64)
    top_i = np.concatenate([np.where(far_top)[0], movers]).astype(np.int64)
    bot_i = np.where(far_bot)[0]
    ordr = np.concatenate([
        keep[np.argsort(rsx[keep][:, 0], kind="stable")],
        top_i[np.argsort(rsx[top_i][:, 0], kind="stable")],
        bot_i[np.argsort(rsx[bot_i][:, 0], kind="stable")],
    ])
    rs2 = rsx[ordr]
    n_near, n_top = len(keep), len(top_i)

    band_t = np.where(pvs[:, 1] >= py_hi - BAND_W)[0]
    band_b = np.where(pvs[:, 1] <= py_lo + BAND_W)[0]
    btx = pvs[band_t][:, 0]
    bbx = pvs[band_b][:, 0]
    BT = _enc_b(pvs[band_t]) if len(band_t) else np.zeros((K, 0), _f16)
    BB = _enc_b(pvs[band_b]) if len(band_b) else np.zeros((K, 0), _f16)

    AT = _enc_a(rs2)
    r_offs = np.zeros(M // 128, np.int64)
    r_kind = [None] * (M // 128)
    for u in range(M // 128):
        lo, hi = 128 * u, 128 * (u + 1)
        if hi <= n_near:
            kind = "near"
        elif lo >= n_near and hi <= n_near + n_top:
            kind = "top"
        elif lo >= n_near + n_top:
            kind = "bot"
        else:
            kind = "top" if (hi - n_near) > 64 and len(btx) else "near"
            if lo >= n_near and len(bbx) and (hi - (n_near + n_top)) > 64:
                kind = "bot"
        r_kind[u] = kind
        cx = {"near": px, "top": btx, "bot": bbx}[kind]
        xlo, xhi = rs2[lo:hi, 0].min(), rs2[lo:hi, 0].max()
        r_offs[u] = _window_offsets(xlo, xhi, cx, len(cx), CR)
    BPm = _enc_b(pvs)
    src = {"near": (BPm, nv), "top": (BT, len(btx)), "bot": (BB, len(bbx))}
    bpg = np.concatenate([
        _gather_windows(src[r_kind[u]][0], src[r_kind[u]][1], [r_offs[u]], CR)
        for u in range(M // 128)
    ], axis=1)

    # ---- run on hardware ----
    in_maps = []
    for cc in range(NCORES):
        ab = np.concatenate([
            A[:, cc * R_pc:(cc + 1) * R_pc],
            bgath[:, cc * T_p * CP:(cc + 1) * T_p * CP],
        ], axis=1)
        cd = np.concatenate([
            AT[:, cc * REF_TILES * 128:(cc + 1) * REF_TILES * 128],
            bpg[:, cc * REF_TILES * CR:(cc + 1) * REF_TILES * CR],
        ], axis=1)
        in_maps.append({"ab_in": np.ascontiguousarray(ab),
                        "cd_in": np.ascontiguousarray(cd)})

    res = _run_on_hw(in_maps, T_p, trace=_trace, tmpdir=_tmpdir)

    rowd2 = np.empty(NP)
    refd2 = np.empty(M)
    for cc in range(NCORES):
        out = res.results[cc]
        rowd2[cc * R_pc:(cc + 1) * R_pc] = \
            out["rowm_out"].astype(np.float64).T.reshape(-1)
        refd2[cc * REF_TILES * 128:(cc + 1) * REF_TILES * 128] = \
            out["refm_out"].astype(np.float64).T.reshape(-1)

    # ---- host: margins, fallback, means ----
    ry_lo, ry_hi = rsx[:, 1].min(), rsx[:, 1].max()
    found1 = np.sqrt(np.maximum(rowd2[:nv], 0.0))
    yc1 = np.maximum(0.0, np.maximum(pvs[:, 1] - ry_hi, ry_lo - pvs[:, 1]))
    marg1 = np.full(nv, np.inf)
    for g in range((nv + 127) // 128):
        lo, hi = 128 * g, min(128 * (g + 1), nv)
        marg1[lo:hi] = _edge_margins(px[lo:hi], yc1[lo:hi], rx, M,
                                     int(p_offs[g]), CP)
    slack1 = np.maximum(1e-3 * found1, 1e-4)
    bad1 = (found1 > marg1 - slack1) | ~np.isfinite(found1)
    if bad1.any():
        ii = np.where(bad1)[0]
        d2x = ((pvs[ii, None, :] - rsx[None, :, :]) ** 2).sum(-1).min(1)
        found1[ii] = np.sqrt(d2x)
    mean1 = found1.mean()

    found2 = np.sqrt(np.maximum(refd2, 0.0))
    yc2 = np.maximum(0.0, np.maximum(rs2[:, 1] - py_hi, py_lo - rs2[:, 1]))
    marg2 = np.full(M, np.inf)
    for u in range(M // 128):
        lo, hi = 128 * u, 128 * (u + 1)
        kind = r_kind[u]
        cx = {"near": px, "top": btx, "bot": bbx}[kind]
        m = _edge_margins(rs2[lo:hi, 0], yc2[lo:hi], cx, len(cx),
                          int(r_offs[u]), CR)
        if kind == "top":
            m = np.minimum(np.maximum(rs2[lo:hi, 1] - (py_hi - BAND_W), 0.0),
                           m)
        elif kind == "bot":
            m = np.minimum(np.maximum((py_lo + BAND_W) - rs2[lo:hi, 1], 0.0),
                           m)
        marg2[lo:hi] = m
    slack2 = np.maximum(1e-3 * found2, 1e-4)
    bad2 = (found2 > marg2 - slack2) | ~np.isfinite(found2)
    if bad2.any():
        jj = np.where(bad2)[0]
        d2x = ((rs2[jj, None, :] - pvs[None, :, :]) ** 2).sum(-1).min(1)
        found2[jj] = np.sqrt(d2x)
    mean2 = found2.mean()

    out = np.float32(0.5 * (mean1 + mean2))
    if _trace:
        return out, res
    return out


# revision 34
# speedup vs baseline: 10.0390x; 1.0640x over previous
"""CenterlineLoss Trainium2 kernel — windowed two-pass nearest-neighbor.

Computes 0.5*(mean1 + mean2) where
  mean1 = mean over valid proj points of distance to nearest ref point
  mean2 = mean over ref points of distance to nearest valid proj point
(reference semantics: ref coords swapped; proj row order irrelevant;
proj validity mask applied to both reductions).

Strategy: the all-pairs [N, M] distance matrix is never materialized.
Host sorts the valid proj points and the refs along x and gathers, for
every 128-point tile, a contiguous candidate window (384 refs per proj
tile, 512 proj per ref tile) from the other (sorted) point set.  The
device computes, per tile, a [128, w] squared-distance block via one
TensorE matmul (K=14 fp16 limb-split encoding, d^2 exact to fp32);
tiles are grouped four-to-a-PSUM-allocation and retired by a single
strided DVE tensor_reduce into per-row minima.  Refs whose y lies
beyond the proj y-extent get their candidates from a boundary band of
proj sorted by x instead.

Correctness does not depend on the windows: the host computes, per
query row, a lower bound on the distance to any EXCLUDED candidate
(x-gap to the window edge, y-clearance to the set extent, band bound).
Rows whose found min does not beat that bound are recomputed exactly
on the host (typically 0-2 rows).  Degenerate inputs (few valid
points) fall back to an exact host computation.
"""

import time

import numpy as np

import concourse.bacc as bacc
import concourse.mybir as mybir
import concourse.tile as tile
from concourse import bass_utils

N = 16384
M = 8192
NCORES = 8
K = 14                      # limb-split contraction depth
CP = 320                    # proj-side candidate window per 128-row tile
CR = 416                    # ref-side candidate window per 128-row tile
P2SCALE = 64.0
R2SCALE = 16.0
BIGVAL = 60000.0            # sentinel d^2 (> any real window min)
CENTER = (320.0, 240.0)
BAND_W = 48.0               # boundary-band depth for far refs
TAU = 2.0                   # y-clearance above which a ref is "far"
REF_TILES = M // 128 // NCORES  # 8 ref tiles per core

_f16 = np.float16


def _split2(v):
    h = v.astype(_f16).astype(np.float64)
    l = (v - h).astype(_f16).astype(np.float64)
    return h, l


def _split3(v):
    h = v.astype(_f16).astype(np.float64)
    r = v - h
    m = r.astype(_f16).astype(np.float64)
    l = (r - m).astype(_f16).astype(np.float64)
    return h, m, l


def _enc_a(pts):
    """Row-side limb encoding (points on the partition axis). [n,2]->[K,n]"""
    x = pts[:, 0]
    y = pts[:, 1]
    Xh, Xl = _split2(x)
    Yh, Yl = _split2(y)
    px, py = Xh + Xl, Yh + Yl
    P2a, P2b, P2c = _split3((px * px + py * py) / P2SCALE)
    rs = np.full(len(x), R2SCALE)
    return np.stack(
        [Xh, Xh, Xl, Xl, Yh, Yh, Yl, Yl, P2a, P2b, P2c, rs, rs, rs]
    ).astype(_f16)


def _enc_b(pts):
    """Column-side limb encoding (candidate points). [n,2]->[K,n]"""
    x = pts[:, 0]
    y = pts[:, 1]
    Xh, Xl = _split2(x)
    Yh, Yl = _split2(y)
    rx, ry = Xh + Xl, Yh + Yl
    R2a, R2b, R2c = _split3((rx * rx + ry * ry) / R2SCALE)
    ps = np.full(len(x), P2SCALE)
    return np.stack(
        [-2 * Xh, -2 * Xl, -2 * Xh, -2 * Xl,
         -2 * Yh, -2 * Yl, -2 * Yh, -2 * Yl,
         ps, ps, ps, R2a, R2b, R2c]
    ).astype(_f16)


_B_SENT = None


def _b_sentinel():
    """Candidate-side sentinel column: d^2 == BIGVAL against any row."""
    global _B_SENT
    if _B_SENT is None:
        col = np.zeros((K, 1), _f16)
        col[11, 0] = _f16(BIGVAL / R2SCALE)
        _B_SENT = col
    return _B_SENT


_PROGRAM_CACHE = {}

# NOTE on rejected variants (hardware constraints discovered on the way):
# GPSIMD cannot run TensorTensor or touch PSUM; DVE may read at most one
# PSUM operand per instruction; TENSOR_TENSOR_REDUCE with op=min crashed
# the exec unit.  A plain strided tensor_reduce from PSUM is both legal
# and the fastest schedule found.


def _build_program(T_p=14):
    key = T_p
    if key in _PROGRAM_CACHE:
        return _PROGRAM_CACHE[key]

    f16 = mybir.dt.float16
    f32 = mybir.dt.float32
    MIN = mybir.AluOpType.min

    nc = bacc.Bacc("TRN2", target_bir_lowering=False, debug=False,
                   num_devices=NCORES)

    WAB = T_p * (128 + CP)
    WCD = REF_TILES * (128 + CR)
    ab_dram = nc.dram_tensor("ab_in", [K, WAB], f16, kind="ExternalInput").ap()
    cd_dram = nc.dram_tensor("cd_in", [K, WCD], f16, kind="ExternalInput").ap()
    rowm_dram = nc.dram_tensor("rowm_out", [128, T_p], f32,
                               kind="ExternalOutput").ap()
    refm_dram = nc.dram_tensor("refm_out", [128, REF_TILES], f32,
                               kind="ExternalOutput").ap()

    with tile.TileContext(nc) as tc, \
            tc.tile_pool(name="const", bufs=1) as cpool:
        ab_sb = cpool.tile([K, WAB], f16, tag="ab")
        cd_sb = cpool.tile([K, WCD], f16, tag="cd")
        rowm = cpool.tile([128, T_p], f32, tag="rowm")
        refm = cpool.tile([128, REF_TILES], f32, tag="refm")

        # inputs on two separate queues so both are in flight immediately
        nc.sync.dma_start(ab_sb[:], ab_dram)
        nc.scalar.dma_start(cd_sb[:], cd_dram)

        # matmul tiles are grouped 4-per-PSUM-allocation (at 512-col
        # stride = one bank per tile) so a single strided tensor_reduce
        # [128, g, w] -> [128, g] retires a whole group.  The expensive
        # ref groups run in the middle; a cheap proj group retires last
        # so the final output DMA tails a short op.
        with tc.tile_pool(name="mm", bufs=2, space="PSUM") as pspool:

            def do_group(tiles, a_base, b_base, w, acc, res_sb):
                g = len(tiles)
                ps = pspool.tile([128, 2048], f32, tag="mm")
                for k, t in enumerate(tiles):
                    nc.tensor.matmul(
                        ps[:, k * 512:k * 512 + w],
                        res_sb[:, a_base + t * 128:a_base + (t + 1) * 128],
                        res_sb[:, b_base + t * w:b_base + (t + 1) * w],
                        start=True, stop=True)
                view = ps[:].rearrange("p (b f) -> p b f", f=512)
                nc.vector.tensor_reduce(acc, view[:, :g, :w], op=MIN,
                                        axis=mybir.AxisListType.X)

            # first group small so the serial reduce chain starts as
            # soon as the input lands; small groups last keep the tail short
            sizes = [2]
            rem = T_p - 2
            while rem > 4:
                sizes.append(4)
                rem -= 4
            while rem > 0:
                sizes.append(min(2, rem))
                rem -= min(2, rem)
            pgroups, t0 = [], 0
            for s in sizes:
                pgroups.append(list(range(t0, t0 + s)))
                t0 += s
            rgroups = [list(range(u0, min(u0 + 4, REF_TILES)))
                       for u0 in range(0, REF_TILES, 4)]
            for kind, tiles in ([("p", t) for t in pgroups[:2]]
                                + [("r", t) for t in rgroups]
                                + [("p", t) for t in pgroups[2:]]):
                if kind == "p":
                    do_group(tiles, 0, T_p * 128, CP,
                             rowm[:, tiles[0]:tiles[0] + len(tiles)], ab_sb)
                else:
                    do_group(tiles, 0, REF_TILES * 128, CR,
                             refm[:, tiles[0]:tiles[0] + len(tiles)], cd_sb)
                    if tiles[0] + len(tiles) == REF_TILES:
                        nc.sync.dma_start(refm_dram, refm[:])
            nc.sync.dma_start(rowm_dram, rowm[:])

    nc.compile()
    _PROGRAM_CACHE[key] = nc
    return nc


def _gather_windows(enc, n_real, offs, w):
    """Stack enc[:, o:o+w] slices; pad short sources with sentinels."""
    cols = []
    for o in offs:
        if n_real >= w:
            cols.append(enc[:, o:o + w])
        else:
            pad = np.broadcast_to(_b_sentinel(), (K, w - n_real))
            cols.append(np.concatenate([enc[:, :n_real], pad], axis=1))
    return np.concatenate(cols, axis=1)


def _window_offsets(tile_lo_x, tile_hi_x, cand_x, n_cand, w):
    ja = np.searchsorted(cand_x, tile_lo_x)
    jb = np.searchsorted(cand_x, tile_hi_x)
    return int(np.clip((ja + jb) // 2 - w // 2, 0, max(0, n_cand - w)))


def _edge_margins(qx, yclear, cand_x, n_cand, o, w):
    """Min distance from query rows to any candidate excluded by the
    x-window [o, o+w) — hypot of x-gap past the nearest excluded
    element and the y-clearance to the candidate set's y-extent."""
    n = len(qx)
    if o > 0:
        ml = np.hypot(np.maximum(qx - cand_x[o - 1], 0.0), yclear)
    else:
        ml = np.full(n, np.inf)
    if o + w < n_cand:
        mr = np.hypot(np.maximum(cand_x[o + w] - qx, 0.0), yclear)
    else:
        mr = np.full(n, np.inf)
    return np.minimum(ml, mr)


def _run_on_hw(in_maps, T_p, trace=False, tmpdir=None):
    nc = _build_program(T_p)
    last = None
    for wait_s in (0, 30, 60, 90):
        if wait_s:
            time.sleep(wait_s)
        try:
            return bass_utils.run_bass_kernel_spmd(
                nc, in_maps, core_ids=list(range(NCORES)), trace=trace,
                tmpdir=tmpdir,
            )
        except Exception as e:
            last = e
    raise last


def kernel(bezier_proj_centerline_img, ref_catheter_centerline, _trace=False,
           _tmpdir=None):
    proj = np.asarray(bezier_proj_centerline_img, np.float64)
    refs_all = np.asarray(ref_catheter_centerline, np.float64)[:, ::-1]
    c = np.array(CENTER)

    mask = (
        (proj[:, 0] >= 0.0) & (proj[:, 0] <= 640.0)
        & (proj[:, 1] >= 0.0) & (proj[:, 1] <= 480.0)
    )
    pv = proj[mask]
    nv = len(pv)
    m_ref = len(refs_all)

    if nv < 2 * CP or m_ref != M:
        # degenerate input: exact host computation
        if nv == 0:
            mean1 = np.nan
            mean2 = np.sqrt(((refs_all[:, None, :] - proj[None, :, :]) ** 2)
                            .sum(-1)).min(1).mean() if len(proj) else np.nan
            out = np.float32(0.5 * (mean1 + mean2))
        else:
            d2 = ((pv[:, None, :] - refs_all[None, :, :]) ** 2).sum(-1)
            mean1 = np.sqrt(d2.min(1)).mean()
            mean2 = np.sqrt(d2.min(0)).mean()
            out = np.float32(0.5 * (mean1 + mean2))
        if _trace:
            return out, None
        return out

    pvs = pv[np.argsort(pv[:, 0], kind="stable")] - c
    px = pvs[:, 0]
    py_lo, py_hi = pvs[:, 1].min(), pvs[:, 1].max()
    rsx = refs_all[np.argsort(refs_all[:, 0], kind="stable")] - c
    rx = rsx[:, 0]

    R_pc = int(np.ceil(nv / (NCORES * 128))) * 128
    NP = NCORES * R_pc
    T_p = R_pc // 128
    T_tot = NP // 128

    # ---- proj-side pass: rows = sorted valid proj, candidates = refs ----
    A = np.concatenate([_enc_a(pvs), np.zeros((K, NP - nv), _f16)], axis=1)
    A[8, nv:] = _f16(BIGVAL / P2SCALE)
    B = _enc_b(rsx)

    p_offs = np.zeros(T_tot, np.int64)
    for g in range(T_tot):
        lo, hi = 128 * g, min(128 * (g + 1), nv)
        if lo >= nv:
            continue
        p_offs[g] = _window_offsets(px[lo], px[hi - 1], rx, M, CP)
    bgath = _gather_windows(B, M, p_offs, CP)

    # ---- ref-side pass: rows = refs (class-ordered), candidates = proj ----
    far_top = rsx[:, 1] > py_hi + TAU
    far_bot = rsx[:, 1] < py_lo - TAU
    near_i = np.where(~(far_top | far_bot))[0]
    n_keep = (len(near_i) // 128) * 128
    if n_keep < len(near_i):
        by_y = near_i[np.argsort(np.abs(rsx[near_i][:, 1]), kind="stable")]
        keep, movers = by_y[:n_keep], by_y[n_keep:]
    else:
        keep, movers = near_i, np.array([], np.int64)
    top_i = np.concatenate([np.where(far_top)[0], movers]).astype(np.int64)
    bot_i = np.where(far_bot)[0]
    ordr = np.concatenate([
        keep[np.argsort(rsx[keep][:, 0], kind="stable")],
        top_i[np.argsort(rsx[top_i][:, 0], kind="stable")],
        bot_i[np.argsort(rsx[bot_i][:, 0], kind="stable")],
    ])
    rs2 = rsx[ordr]
    n_near, n_top = len(keep), len(top_i)

    band_t = np.where(pvs[:, 1] >= py_hi - BAND_W)[0]
    band_b = np.where(pvs[:, 1] <= py_lo + BAND_W)[0]
    btx = pvs[band_t][:, 0]
    bbx = pvs[band_b][:, 0]
    BT = _enc_b(pvs[band_t]) if len(band_t) else np.zeros((K, 0), _f16)
    BB = _enc_b(pvs[band_b]) if len(band_b) else np.zeros((K, 0), _f16)

    AT = _enc_a(rs2)
    r_offs = np.zeros(M // 128, np.int64)
    r_kind = [None] * (M // 128)
    for u in range(M // 128):
        lo, hi = 128 * u, 128 * (u + 1)
        if hi <= n_near:
            kind = "near"
        elif lo >= n_near and hi <= n_near + n_top:
            kind = "top"
        elif lo >= n_near + n_top:
            kind = "bot"
        else:
            kind = "top" if (hi - n_near) > 64 and len(btx) else "near"
            if lo >= n_near and len(bbx) and (hi - (n_near + n_top)) > 64:
                kind = "bot"
        r_kind[u] = kind
        cx = {"near": px, "top": btx, "bot": bbx}[kind]
        xlo, xhi = rs2[lo:hi, 0].min(), rs2[lo:hi, 0].max()
        r_offs[u] = _window_offsets(xlo, xhi, cx, len(cx), CR)
    BPm = _enc_b(pvs)
    src = {"near": (BPm, nv), "top": (BT, len(btx)), "bot": (BB, len(bbx))}
    bpg = np.concatenate([
        _gather_windows(src[r_kind[u]][0], src[r_kind[u]][1], [r_offs[u]], CR)
        for u in range(M // 128)
    ], axis=1)

    # ---- run on hardware ----
    in_maps = []
    for cc in range(NCORES):
        ab = np.concatenate([
            A[:, cc * R_pc:(cc + 1) * R_pc],
            bgath[:, cc * T_p * CP:(cc + 1) * T_p * CP],
        ], axis=1)
        cd = np.concatenate([
            AT[:, cc * REF_TILES * 128:(cc + 1) * REF_TILES * 128],
            bpg[:, cc * REF_TILES * CR:(cc + 1) * REF_TILES * CR],
        ], axis=1)
        in_maps.append({"ab_in": np.ascontiguousarray(ab),
                        "cd_in": np.ascontiguousarray(cd)})

    res = _run_on_hw(in_maps, T_p, trace=_trace, tmpdir=_tmpdir)

    rowd2 = np.empty(NP)
    refd2 = np.empty(M)
    for cc in range(NCORES):
        out = res.results[cc]
        rowd2[cc * R_pc:(cc + 1) * R_pc] = \
            out["rowm_out"].astype(np.float64).T.reshape(-1)
        refd2[cc * REF_TILES * 128:(cc + 1) * REF_TILES * 128] = \
            out["refm_out"].astype(np.float64).T.reshape(-1)

    # ---- host: margins, fallback, means ----
    ry_lo, ry_hi = rsx[:, 1].min(), rsx[:, 1].max()
    found1 = np.sqrt(np.maximum(rowd2[:nv], 0.0))
    yc1 = np.maximum(0.0, np.maximum(pvs[:, 1] - ry_hi, ry_lo - pvs[:, 1]))
    marg1 = np.full(nv, np.inf)
    for g in range((nv + 127) // 128):
        lo, hi = 128 * g, min(128 * (g + 1), nv)
        marg1[lo:hi] = _edge_margins(px[lo:hi], yc1[lo:hi], rx, M,
                                     int(p_offs[g]), CP)
    slack1 = np.maximum(1e-3 * found1, 1e-4)
    bad1 = (found1 > marg1 - slack1) | ~np.isfinite(found1)
    if bad1.any():
        ii = np.where(bad1)[0]
        d2x = ((pvs[ii, None, :] - rsx[None, :, :]) ** 2).sum(-1).min(1)
        found1[ii] = np.sqrt(d2x)
    mean1 = found1.mean()

    found2 = np.sqrt(np.maximum(refd2, 0.0))
    yc2 = np.maximum(0.0, np.maximum(rs2[:, 1] - py_hi, py_lo - rs2[:, 1]))
    marg2 = np.full(M, np.inf)
    for u in range(M // 128):
        lo, hi = 128 * u, 128 * (u + 1)
        kind = r_kind[u]
        cx = {"near": px, "top": btx, "bot": bbx}[kind]
        m = _edge_margins(rs2[lo:hi, 0], yc2[lo:hi], cx, len(cx),
                          int(r_offs[u]), CR)
        if kind == "top":
            m = np.minimum(np.maximum(rs2[lo:hi, 1] - (py_hi - BAND_W), 0.0),
                           m)
        elif kind == "bot":
            m = np.minimum(np.maximum((py_lo + BAND_W) - rs2[lo:hi, 1], 0.0),
                           m)
        marg2[lo:hi] = m
    slack2 = np.maximum(1e-3 * found2, 1e-4)
    bad2 = (found2 > marg2 - slack2) | ~np.isfinite(found2)
    if bad2.any():
        jj = np.where(bad2)[0]
        d2x = ((rs2[jj, None, :] - pvs[None, :, :]) ** 2).sum(-1).min(1)
        found2[jj] = np.sqrt(d2x)
    mean2 = found2.mean()

    out = np.float32(0.5 * (mean1 + mean2))
    if _trace:
        return out, res
    return out


# revision 35
# speedup vs baseline: 10.4933x; 1.0453x over previous
"""CenterlineLoss Trainium2 kernel — windowed two-pass nearest-neighbor.

Computes 0.5*(mean1 + mean2) where
  mean1 = mean over valid proj points of distance to nearest ref point
  mean2 = mean over ref points of distance to nearest valid proj point
(reference semantics: ref coords swapped; proj row order irrelevant;
proj validity mask applied to both reductions).

Strategy: the all-pairs [N, M] distance matrix is never materialized.
Host sorts the valid proj points and the refs along x and gathers, for
every 128-point tile, a contiguous candidate window (384 refs per proj
tile, 512 proj per ref tile) from the other (sorted) point set.  The
device computes, per tile, a [128, w] squared-distance block via one
TensorE matmul (K=14 fp16 limb-split encoding, d^2 exact to fp32);
tiles are grouped four-to-a-PSUM-allocation and retired by a single
strided DVE tensor_reduce into per-row minima.  Refs whose y lies
beyond the proj y-extent get their candidates from a boundary band of
proj sorted by x instead.

Correctness does not depend on the windows: the host computes, per
query row, a lower bound on the distance to any EXCLUDED candidate
(x-gap to the window edge, y-clearance to the set extent, band bound).
Rows whose found min does not beat that bound are recomputed exactly
on the host (typically 0-2 rows).  Degenerate inputs (few valid
points) fall back to an exact host computation.
"""

import time

import numpy as np

import concourse.bacc as bacc
import concourse.mybir as mybir
import concourse.tile as tile
from concourse import bass_utils

N = 16384
M = 8192
NCORES = 8
K = 14                      # limb-split contraction depth
CP = 288                    # proj-side candidate window per 128-row tile
CR = 400                    # ref-side candidate window per 128-row tile
P2SCALE = 64.0
R2SCALE = 16.0
BIGVAL = 60000.0            # sentinel d^2 (> any real window min)
CENTER = (320.0, 240.0)
BAND_W = 48.0               # boundary-band depth for far refs
TAU = 2.0                   # y-clearance above which a ref is "far"
REF_TILES = M // 128 // NCORES  # 8 ref tiles per core

_f16 = np.float16


def _split2(v):
    h = v.astype(_f16).astype(np.float64)
    l = (v - h).astype(_f16).astype(np.float64)
    return h, l


def _split3(v):
    h = v.astype(_f16).astype(np.float64)
    r = v - h
    m = r.astype(_f16).astype(np.float64)
    l = (r - m).astype(_f16).astype(np.float64)
    return h, m, l


def _enc_a(pts):
    """Row-side limb encoding (points on the partition axis). [n,2]->[K,n]"""
    x = pts[:, 0]
    y = pts[:, 1]
    Xh, Xl = _split2(x)
    Yh, Yl = _split2(y)
    px, py = Xh + Xl, Yh + Yl
    P2a, P2b, P2c = _split3((px * px + py * py) / P2SCALE)
    rs = np.full(len(x), R2SCALE)
    return np.stack(
        [Xh, Xh, Xl, Xl, Yh, Yh, Yl, Yl, P2a, P2b, P2c, rs, rs, rs]
    ).astype(_f16)


def _enc_b(pts):
    """Column-side limb encoding (candidate points). [n,2]->[K,n]"""
    x = pts[:, 0]
    y = pts[:, 1]
    Xh, Xl = _split2(x)
    Yh, Yl = _split2(y)
    rx, ry = Xh + Xl, Yh + Yl
    R2a, R2b, R2c = _split3((rx * rx + ry * ry) / R2SCALE)
    ps = np.full(len(x), P2SCALE)
    return np.stack(
        [-2 * Xh, -2 * Xl, -2 * Xh, -2 * Xl,
         -2 * Yh, -2 * Yl, -2 * Yh, -2 * Yl,
         ps, ps, ps, R2a, R2b, R2c]
    ).astype(_f16)


_B_SENT = None


def _b_sentinel():
    """Candidate-side sentinel column: d^2 == BIGVAL against any row."""
    global _B_SENT
    if _B_SENT is None:
        col = np.zeros((K, 1), _f16)
        col[11, 0] = _f16(BIGVAL / R2SCALE)
        _B_SENT = col
    return _B_SENT


_PROGRAM_CACHE = {}

# NOTE on rejected variants (hardware constraints discovered on the way):
# GPSIMD cannot run TensorTensor or touch PSUM; DVE may read at most one
# PSUM operand per instruction; TENSOR_TENSOR_REDUCE with op=min crashed
# the exec unit.  A plain strided tensor_reduce from PSUM is both legal
# and the fastest schedule found.


def _build_program(T_p=14):
    key = T_p
    if key in _PROGRAM_CACHE:
        return _PROGRAM_CACHE[key]

    f16 = mybir.dt.float16
    f32 = mybir.dt.float32
    MIN = mybir.AluOpType.min

    nc = bacc.Bacc("TRN2", target_bir_lowering=False, debug=False,
                   num_devices=NCORES)

    WAB = T_p * (128 + CP)
    WCD = REF_TILES * (128 + CR)
    ab_dram = nc.dram_tensor("ab_in", [K, WAB], f16, kind="ExternalInput").ap()
    cd_dram = nc.dram_tensor("cd_in", [K, WCD], f16, kind="ExternalInput").ap()
    rowm_dram = nc.dram_tensor("rowm_out", [128, T_p], f32,
                               kind="ExternalOutput").ap()
    refm_dram = nc.dram_tensor("refm_out", [128, REF_TILES], f32,
                               kind="ExternalOutput").ap()

    with tile.TileContext(nc) as tc, \
            tc.tile_pool(name="const", bufs=1) as cpool:
        ab_sb = cpool.tile([K, WAB], f16, tag="ab")
        cd_sb = cpool.tile([K, WCD], f16, tag="cd")
        rowm = cpool.tile([128, T_p], f32, tag="rowm")
        refm = cpool.tile([128, REF_TILES], f32, tag="refm")

        # inputs on two separate queues so both are in flight immediately
        nc.sync.dma_start(ab_sb[:], ab_dram)
        nc.scalar.dma_start(cd_sb[:], cd_dram)

        # matmul tiles are grouped 4-per-PSUM-allocation (at 512-col
        # stride = one bank per tile) so a single strided tensor_reduce
        # [128, g, w] -> [128, g] retires a whole group.  The expensive
        # ref groups run in the middle; a cheap proj group retires last
        # so the final output DMA tails a short op.
        with tc.tile_pool(name="mm", bufs=2, space="PSUM") as pspool:

            def do_group(tiles, a_base, b_base, w, acc, res_sb):
                g = len(tiles)
                ps = pspool.tile([128, 2048], f32, tag="mm")
                for k, t in enumerate(tiles):
                    nc.tensor.matmul(
                        ps[:, k * 512:k * 512 + w],
                        res_sb[:, a_base + t * 128:a_base + (t + 1) * 128],
                        res_sb[:, b_base + t * w:b_base + (t + 1) * w],
                        start=True, stop=True)
                view = ps[:].rearrange("p (b f) -> p b f", f=512)
                nc.vector.tensor_reduce(acc, view[:, :g, :w], op=MIN,
                                        axis=mybir.AxisListType.X)

            # first group small so the serial reduce chain starts as
            # soon as the input lands; small groups last keep the tail short
            sizes = [2]
            rem = T_p - 2
            while rem > 4:
                sizes.append(4)
                rem -= 4
            while rem > 0:
                sizes.append(min(2, rem))
                rem -= min(2, rem)
            pgroups, t0 = [], 0
            for s in sizes:
                pgroups.append(list(range(t0, t0 + s)))
                t0 += s
            rgroups = [list(range(u0, min(u0 + 4, REF_TILES)))
                       for u0 in range(0, REF_TILES, 4)]
            for kind, tiles in ([("p", t) for t in pgroups[:2]]
                                + [("r", t) for t in rgroups]
                                + [("p", t) for t in pgroups[2:]]):
                if kind == "p":
                    do_group(tiles, 0, T_p * 128, CP,
                             rowm[:, tiles[0]:tiles[0] + len(tiles)], ab_sb)
                else:
                    do_group(tiles, 0, REF_TILES * 128, CR,
                             refm[:, tiles[0]:tiles[0] + len(tiles)], cd_sb)
                    if tiles[0] + len(tiles) == REF_TILES:
                        nc.sync.dma_start(refm_dram, refm[:])
            nc.sync.dma_start(rowm_dram, rowm[:])

    nc.compile()
    _PROGRAM_CACHE[key] = nc
    return nc


def _gather_windows(enc, n_real, offs, w):
    """Stack enc[:, o:o+w] slices; pad short sources with sentinels."""
    cols = []
    for o in offs:
        if n_real >= w:
            cols.append(enc[:, o:o + w])
        else:
            pad = np.broadcast_to(_b_sentinel(), (K, w - n_real))
            cols.append(np.concatenate([enc[:, :n_real], pad], axis=1))
    return np.concatenate(cols, axis=1)


def _window_offsets(tile_lo_x, tile_hi_x, cand_x, n_cand, w):
    ja = np.searchsorted(cand_x, tile_lo_x)
    jb = np.searchsorted(cand_x, tile_hi_x)
    return int(np.clip((ja + jb) // 2 - w // 2, 0, max(0, n_cand - w)))


def _edge_margins(qx, yclear, cand_x, n_cand, o, w):
    """Min distance from query rows to any candidate excluded by the
    x-window [o, o+w) — hypot of x-gap past the nearest excluded
    element and the y-clearance to the candidate set's y-extent."""
    n = len(qx)
    if o > 0:
        ml = np.hypot(np.maximum(qx - cand_x[o - 1], 0.0), yclear)
    else:
        ml = np.full(n, np.inf)
    if o + w < n_cand:
        mr = np.hypot(np.maximum(cand_x[o + w] - qx, 0.0), yclear)
    else:
        mr = np.full(n, np.inf)
    return np.minimum(ml, mr)


def _run_on_hw(in_maps, T_p, trace=False, tmpdir=None):
    nc = _build_program(T_p)
    last = None
    for wait_s in (0, 30, 60, 90):
        if wait_s:
            time.sleep(wait_s)
        try:
            return bass_utils.run_bass_kernel_spmd(
                nc, in_maps, core_ids=list(range(NCORES)), trace=trace,
                tmpdir=tmpdir,
            )
        except Exception as e:
            last = e
    raise last


def kernel(bezier_proj_centerline_img, ref_catheter_centerline, _trace=False,
           _tmpdir=None):
    proj = np.asarray(bezier_proj_centerline_img, np.float64)
    refs_all = np.asarray(ref_catheter_centerline, np.float64)[:, ::-1]
    c = np.array(CENTER)

    mask = (
        (proj[:, 0] >= 0.0) & (proj[:, 0] <= 640.0)
        & (proj[:, 1] >= 0.0) & (proj[:, 1] <= 480.0)
    )
    pv = proj[mask]
    nv = len(pv)
    m_ref = len(refs_all)

    if nv < 2 * CP or m_ref != M:
        # degenerate input: exact host computation
        if nv == 0:
            mean1 = np.nan
            mean2 = np.sqrt(((refs_all[:, None, :] - proj[None, :, :]) ** 2)
                            .sum(-1)).min(1).mean() if len(proj) else np.nan
            out = np.float32(0.5 * (mean1 + mean2))
        else:
            d2 = ((pv[:, None, :] - refs_all[None, :, :]) ** 2).sum(-1)
            mean1 = np.sqrt(d2.min(1)).mean()
            mean2 = np.sqrt(d2.min(0)).mean()
            out = np.float32(0.5 * (mean1 + mean2))
        if _trace:
            return out, None
        return out

    pvs = pv[np.argsort(pv[:, 0], kind="stable")] - c
    px = pvs[:, 0]
    py_lo, py_hi = pvs[:, 1].min(), pvs[:, 1].max()
    rsx = refs_all[np.argsort(refs_all[:, 0], kind="stable")] - c
    rx = rsx[:, 0]

    R_pc = int(np.ceil(nv / (NCORES * 128))) * 128
    NP = NCORES * R_pc
    T_p = R_pc // 128
    T_tot = NP // 128

    # ---- proj-side pass: rows = sorted valid proj, candidates = refs ----
    A = np.concatenate([_enc_a(pvs), np.zeros((K, NP - nv), _f16)], axis=1)
    A[8, nv:] = _f16(BIGVAL / P2SCALE)
    B = _enc_b(rsx)

    p_offs = np.zeros(T_tot, np.int64)
    for g in range(T_tot):
        lo, hi = 128 * g, min(128 * (g + 1), nv)
        if lo >= nv:
            continue
        p_offs[g] = _window_offsets(px[lo], px[hi - 1], rx, M, CP)
    bgath = _gather_windows(B, M, p_offs, CP)

    # ---- ref-side pass: rows = refs (class-ordered), candidates = proj ----
    far_top = rsx[:, 1] > py_hi + TAU
    far_bot = rsx[:, 1] < py_lo - TAU
    near_i = np.where(~(far_top | far_bot))[0]
    n_keep = (len(near_i) // 128) * 128
    if n_keep < len(near_i):
        by_y = near_i[np.argsort(np.abs(rsx[near_i][:, 1]), kind="stable")]
        keep, movers = by_y[:n_keep], by_y[n_keep:]
    else:
        keep, movers = near_i, np.array([], np.int64)
    top_i = np.concatenate([np.where(far_top)[0], movers]).astype(np.int64)
    bot_i = np.where(far_bot)[0]
    ordr = np.concatenate([
        keep[np.argsort(rsx[keep][:, 0], kind="stable")],
        top_i[np.argsort(rsx[top_i][:, 0], kind="stable")],
        bot_i[np.argsort(rsx[bot_i][:, 0], kind="stable")],
    ])
    rs2 = rsx[ordr]
    n_near, n_top = len(keep), len(top_i)

    band_t = np.where(pvs[:, 1] >= py_hi - BAND_W)[0]
    band_b = np.where(pvs[:, 1] <= py_lo + BAND_W)[0]
    btx = pvs[band_t][:, 0]
    bbx = pvs[band_b][:, 0]
    BT = _enc_b(pvs[band_t]) if len(band_t) else np.zeros((K, 0), _f16)
    BB = _enc_b(pvs[band_b]) if len(band_b) else np.zeros((K, 0), _f16)

    AT = _enc_a(rs2)
    r_offs = np.zeros(M // 128, np.int64)
    r_kind = [None] * (M // 128)
    for u in range(M // 128):
        lo, hi = 128 * u, 128 * (u + 1)
        if hi <= n_near:
            kind = "near"
        elif lo >= n_near and hi <= n_near + n_top:
            kind = "top"
        elif lo >= n_near + n_top:
            kind = "bot"
        else:
            kind = "top" if (hi - n_near) > 64 and len(btx) else "near"
            if lo >= n_near and len(bbx) and (hi - (n_near + n_top)) > 64:
                kind = "bot"
        r_kind[u] = kind
        cx = {"near": px, "top": btx, "bot": bbx}[kind]
        xlo, xhi = rs2[lo:hi, 0].min(), rs2[lo:hi, 0].max()
        r_offs[u] = _window_offsets(xlo, xhi, cx, len(cx), CR)
    BPm = _enc_b(pvs)
    src = {"near": (BPm, nv), "top": (BT, len(btx)), "bot": (BB, len(bbx))}
    bpg = np.concatenate([
        _gather_windows(src[r_kind[u]][0], src[r_kind[u]][1], [r_offs[u]], CR)
        for u in range(M // 128)
    ], axis=1)

    # ---- run on hardware ----
    in_maps = []
    for cc in range(NCORES):
        ab = np.concatenate([
            A[:, cc * R_pc:(cc + 1) * R_pc],
            bgath[:, cc * T_p * CP:(cc + 1) * T_p * CP],
        ], axis=1)
        cd = np.concatenate([
            AT[:, cc * REF_TILES * 128:(cc + 1) * REF_TILES * 128],
            bpg[:, cc * REF_TILES * CR:(cc + 1) * REF_TILES * CR],
        ], axis=1)
        in_maps.append({"ab_in": np.ascontiguousarray(ab),
                        "cd_in": np.ascontiguousarray(cd)})

    res = _run_on_hw(in_maps, T_p, trace=_trace, tmpdir=_tmpdir)

    rowd2 = np.empty(NP)
    refd2 = np.empty(M)
    for cc in range(NCORES):
        out = res.results[cc]
        rowd2[cc * R_pc:(cc + 1) * R_pc] = \
            out["rowm_out"].astype(np.float64).T.reshape(-1)
        refd2[cc * REF_TILES * 128:(cc + 1) * REF_TILES * 128] = \
            out["refm_out"].astype(np.float64).T.reshape(-1)

    # ---- host: margins, fallback, means ----
    ry_lo, ry_hi = rsx[:, 1].min(), rsx[:, 1].max()
    found1 = np.sqrt(np.maximum(rowd2[:nv], 0.0))
    yc1 = np.maximum(0.0, np.maximum(pvs[:, 1] - ry_hi, ry_lo - pvs[:, 1]))
    marg1 = np.full(nv, np.inf)
    for g in range((nv + 127) // 128):
        lo, hi = 128 * g, min(128 * (g + 1), nv)
        marg1[lo:hi] = _edge_margins(px[lo:hi], yc1[lo:hi], rx, M,
                                     int(p_offs[g]), CP)
    slack1 = np.maximum(1e-3 * found1, 1e-4)
    bad1 = (found1 > marg1 - slack1) | ~np.isfinite(found1)
    if bad1.any():
        ii = np.where(bad1)[0]
        d2x = ((pvs[ii, None, :] - rsx[None, :, :]) ** 2).sum(-1).min(1)
        found1[ii] = np.sqrt(d2x)
    mean1 = found1.mean()

    found2 = np.sqrt(np.maximum(refd2, 0.0))
    yc2 = np.maximum(0.0, np.maximum(rs2[:, 1] - py_hi, py_lo - rs2[:, 1]))
    marg2 = np.full(M, np.inf)
    for u in range(M // 128):
        lo, hi = 128 * u, 128 * (u + 1)
        kind = r_kind[u]
        cx = {"near": px, "top": btx, "bot": bbx}[kind]
        m = _edge_margins(rs2[lo:hi, 0], yc2[lo:hi], cx, len(cx),
                          int(r_offs[u]), CR)
        if kind == "top":
            m = np.minimum(np.maximum(rs2[lo:hi, 1] - (py_hi - BAND_W), 0.0),
                           m)
        elif kind == "bot":
            m = np.minimum(np.maximum((py_lo + BAND_W) - rs2[lo:hi, 1], 0.0),
                           m)
        marg2[lo:hi] = m
    slack2 = np.maximum(1e-3 * found2, 1e-4)
    bad2 = (found2 > marg2 - slack2) | ~np.isfinite(found2)
    if bad2.any():
        jj = np.where(bad2)[0]
        d2x = ((rs2[jj, None, :] - pvs[None, :, :]) ** 2).sum(-1).min(1)
        found2[jj] = np.sqrt(d2x)
    mean2 = found2.mean()

    out = np.float32(0.5 * (mean1 + mean2))
    if _trace:
        return out, res
    return out
